# revision 45
# baseline (speedup 1.0000x reference)
"""Trainium2 Bass kernel for nn_AttentionTD (3-block deformable attention TD).

Self-contained: hardcodes all shapes. Data-parallel over batch B=8 across the
8 NeuronCores; each core runs the full 3-block DAT stack for one batch element.

Pipelined emission: block b+1's front-end (q-proj, offset conv, LN, GELU,
index math, kv gather/proj) is interleaved under block b's attention so the
tensor engine never drains between blocks.
"""

import sys

sys.path.insert(0, "/opt/trn_rl_repo")

import numpy as np

# ---------------- problem constants ----------------
B, C, H, W = 8, 128, 64, 64
NCH = 64          # channels per DAT block
NH, HC = 4, 16    # heads, head channels
KS = 4
HWS = H * W       # 4096
HK = WK = 16
NS = HK * WK      # 256 sample points
EPS = 1e-5
NBLK = 3
# rpe slice table geometry: [blk][h][x0 (64)][row (128)][col (65)]
TROW, TCOL = 128, 65
TSLICE = TROW * TCOL          # 8320
THEAD = 64 * TSLICE           # per (blk,h)
TBLK = NH * THEAD
NTAB = NBLK * TBLK

_CACHE = {}


def _build_graph():
    from concourse import bacc, mybir, tile
    import concourse.bass as bass
    from concourse.bass import IndirectOffsetOnAxis

    f32 = mybir.dt.float32
    bf16 = mybir.dt.bfloat16
    i32 = mybir.dt.int32
    Alu = mybir.AluOpType
    Act = mybir.ActivationFunctionType

    nc = bacc.Bacc("TRN2", target_bir_lowering=False, debug=False, num_devices=8)

    # ---- dram io ----
    xi1_d = nc.dram_tensor("xi1", [C, HWS], f32, kind="ExternalInput").ap()
    xi2_d = nc.dram_tensor("xi2", [C, HWS], f32, kind="ExternalInput").ap()
    kvT0_d = nc.dram_tensor("kvT0", [HWS, NCH], f32, kind="ExternalInput").ap()
    kvT1_d = nc.dram_tensor("kvT1", [HWS, NCH], f32, kind="ExternalInput").ap()
    xq1_d = nc.dram_tensor("xq1", [64, HWS], mybir.dt.float32r, kind="ExternalInput").ap()
    xq2_d = nc.dram_tensor("xq2", [64, HWS], mybir.dt.float32r, kind="ExternalInput").ap()
    wpf_d = nc.dram_tensor("wpf", [64, 3 * 128], mybir.dt.float32r, kind="ExternalInput").ap()
    wpb_d = nc.dram_tensor("wpb", [65, 3 * 192], bf16, kind="ExternalInput").ap()
    cp_d = nc.dram_tensor("cp", [128, 599], f32, kind="ExternalInput").ap()
    cpb_d = nc.dram_tensor("cpb", [128, 320], bf16, kind="ExternalInput").ap()
    tab_d = nc.dram_tensor("rpetab", [2 * NTAB, 1], bf16, kind="ExternalInput").ap()
    o1_d = nc.dram_tensor("o1", [C, HWS], f32, kind="ExternalOutput").ap()
    o2_d = nc.dram_tensor("o2", [C, HWS], f32, kind="ExternalOutput").ap()

    with tile.TileContext(nc) as tc:
        import contextlib

        ctx = contextlib.ExitStack()
        with ctx:
            cpool = ctx.enter_context(tc.tile_pool(name="const", bufs=1))
            xpool = ctx.enter_context(tc.tile_pool(name="xdata", bufs=1))
            qpool = ctx.enter_context(tc.tile_pool(name="qtiles", bufs=2))
            wpool = ctx.enter_context(tc.tile_pool(name="wins", bufs=2))
            ppool = ctx.enter_context(tc.tile_pool(name="probs", bufs=1))
            apool = ctx.enter_context(tc.tile_pool(name="avsp", bufs=2))
            spool = ctx.enter_context(tc.tile_pool(name="small", bufs=2))
            qkps = ctx.enter_context(tc.tile_pool(name="qk", bufs=2, space="PSUM"))
            tlps = ctx.enter_context(tc.tile_pool(name="tl", bufs=3, space="PSUM"))
            mps = ctx.enter_context(tc.tile_pool(name="misc", bufs=1, space="PSUM"))

            # ---- persistent loads ----
            cp = cpool.tile([128, 599], f32, tag="cp")
            nc.sync.dma_start(out=cp[:, :], in_=cp_d)
            wpf = cpool.tile([64, 3 * 128], mybir.dt.float32r, tag="wpf")
            nc.sync.dma_start(out=wpf[:, :], in_=wpf_d)
            wpb = cpool.tile([65, 3 * 192], bf16, tag="wpb")
            nc.sync.dma_start(out=wpb[:, :], in_=wpb_d)
            cpb = cpool.tile([128, 320], bf16, tag="cpb")
            xq2 = xpool.tile([64, HWS], mybir.dt.float32r, tag="xq2")
            xr1 = xpool.tile([64, HWS], f32, tag="xr1")
            xr2 = xpool.tile([64, HWS], f32, tag="xr2")

            def load_bulk():
                nc.sync.dma_start(out=cpb[:, :], in_=cpb_d)
                nc.sync.dma_start(out=xq2[:, :], in_=xq2_d)
                nc.sync.dma_start(out=xr1[:, :], in_=xi1_d[64:128, :])
                nc.sync.dma_start(out=xr2[:, :], in_=xi2_d[64:128, :])


            def act_raw(out, in_, func):
                eng = nc.scalar
                ins = [eng.lower_ap(in_)]
                for v in (0.0, 1.0, 0.0):
                    ins.append(mybir.ImmediateValue(dtype=mybir.dt.float32, value=v))
                return eng.add_instruction(
                    mybir.InstActivation(
                        name=nc.get_next_instruction_name(), func=func,
                        ins=ins, outs=[eng.lower_ap(out)],
                    )
                )

            zb = cpool.tile([128, 1], f32, tag="zb")
            nc.vector.memset(zb[:, :], 0.0)
            epst = cpool.tile([1, 1], f32, tag="epst")
            nc.vector.memset(epst[:, :], EPS)
            # vT1 template: zeros with 1.0 at (c*128 + h*32 + 16)
            vtm = cpool.tile([128, 256], bf16, tag="vtm")
            nc.vector.memset(vtm[:, :], 0.0)
            nc.vector.memset(
                vtm[:, :].rearrange("p (c h q) -> p c h q", c=2, q=32)[:, :, :, 16:17],
                1.0,
            )

            eye = cp[:, 0:128]
            ref_yx = cp[0:2, 128:384]          # row0 = y, row1 = x
            ones1_128 = cp[0:1, 384:512]       # [1,128] ones (bcast lhsT)
            ones128_div = cp[0:128, 520:521]   # 1/64 on data rows, 0 on gaps
            kvoff4 = cp[:, 590:594]            # (0,1,64,65) rows
            headoff4 = cp[:, 594:598]          # (0,T,2T,3T) rows
            ones16 = cp[:, 598:599]            # 1.0 at rows h*32+16


            def issue_wpair(idxsrc, h, c):
                Wt = wpool.tile([128, 4160], bf16, tag="Wt", bufs=2, name="Wt")
                nc.gpsimd.indirect_dma_start(
                    out=Wt[:, :], out_offset=None, in_=tab_d,
                    in_offset=IndirectOffsetOnAxis(
                        ap=idxsrc[:, c * 4 + h : c * 4 + h + 1], axis=0),
                )
                Dw = wpool.tile([128, 4160], bf16, tag="Dw", bufs=2, name="Dw")
                nc.gpsimd.indirect_dma_start(
                    out=Dw[:, :], out_offset=None, in_=tab_d,
                    in_offset=IndirectOffsetOnAxis(
                        ap=idxsrc[:, c * 4 + h : c * 4 + h + 1], axis=0),
                    element_offset=NTAB,
                )
                return Wt, Dw

            # ======================= front-end =======================
            def front(blk, XQb, kvT_ap, fc, stream_d=None):
                bc0 = 527 + blk * 21
                dw_w = cp[:, bc0 : bc0 + 16]
                dw_b = cp[:, bc0 + 16 : bc0 + 17]
                ln_g = cp[:, bc0 + 17 : bc0 + 18]
                ln_b = cp[:, bc0 + 18 : bc0 + 19]
                pw_wT = cp[:, bc0 + 19 : bc0 + 21]
                pq_b_sp = cp[:, 521 + blk : 522 + blk]

                # ---- q projection (f32 copy for conv path + bf16 for QK) ----
                q_f = qpool.tile([128, HWS], mybir.dt.float32r, tag="qf", bufs=1)
                q_b = qpool.tile([128, HWS], bf16, tag="qb")
                f32r = mybir.dt.float32r
                for kq in range(4):
                    if stream_d is not None:
                        xqc = spool.tile([64, 1024], mybir.dt.float32r, tag="xqc")
                        nc.sync.dma_start(out=xqc[:, :],
                                          in_=stream_d[:, kq * 1024 : (kq + 1) * 1024])
                        rhs_kq = xqc[:, :]
                    else:
                        rhs_kq = XQb[:, kq * 1024 : (kq + 1) * 1024]
                    qp = qkps.tile([128, 1024], f32, tag="qk")
                    for hf in range(2):
                        nc.tensor.matmul(
                            out=qp[:, hf * 512 : (hf + 1) * 512],
                            lhsT=wpf[:, blk * 128 : (blk + 1) * 128],
                            rhs=rhs_kq[:, hf * 512 : (hf + 1) * 512],
                            start=True, stop=True,
                        )
                    for hf in range(2):
                        nc.scalar.activation(
                            out=q_f[:, kq * 1024 + hf * 512 : kq * 1024 + (hf + 1) * 512],
                            in_=qp[:, hf * 512 : (hf + 1) * 512], func=Act.Copy)
                    nc.vector.tensor_scalar(
                        out=q_b[:, kq * 1024 : (kq + 1) * 1024], in0=qp[:, :],
                        scalar1=ones16, scalar2=None, op0=Alu.add,
                    )
                    if kq == 1:
                        yield
                fc["q_b"] = q_b
                yield
                yield

                # ---- depthwise 4x4 stride-4 conv (f32r diag matmuls) ----
                q5 = q_f[:, :].rearrange("p (hh a ww b) -> p hh a ww b", hh=16, a=4, ww=16, b=4)
                dwp = mps.tile([128, 256], f32, tag="m")
                for t in range(16):
                    dy, dx = t // 4, t % 4
                    dg = spool.tile([128, 128], mybir.dt.float32r, tag="dg", bufs=4)
                    nc.vector.tensor_scalar(out=dg[:, :], in0=eye,
                                            scalar1=dw_w[:, t : t + 1],
                                            scalar2=None, op0=Alu.mult)
                    nc.tensor.matmul(
                        out=dwp[:, :], lhsT=dg[:, :],
                        rhs=q5[:, :, dy, :, dx],
                        start=(t == 0), stop=(t == 15),
                    )
                    if t == 8:
                        yield
                accp = spool.tile([128, 512], f32, tag="accp", bufs=1)
                acc = accp[:, 0:256]
                nc.vector.tensor_scalar(out=acc, in0=dwp[:, :], scalar1=dw_b,
                                        scalar2=None, op0=Alu.add)
                nc.vector.tensor_tensor(out=accp[:, 256:512], in0=acc,
                                        in1=acc, op=Alu.mult)
                yield

                # ---- layernorm stats ----
                stp = mps.tile([1, 512], f32, tag="m")
                nc.tensor.matmul(out=stp[:, :], lhsT=ones128_div, rhs=accp[:, :],
                                 start=True, stop=True)
                stats = spool.tile([1, 512], f32, tag="stats")
                nc.vector.tensor_copy(out=stats[:, 0:256], in_=stp[:, 0:256])
                var = spool.tile([1, 256], f32, tag="var")
                nc.vector.tensor_tensor(out=var[:, :], in0=stats[:, 0:256],
                                        in1=stats[:, 0:256], op=Alu.mult)
                nc.vector.tensor_tensor(out=var[:, :], in0=stp[:, 256:512],
                                        in1=var[:, :], op=Alu.subtract)
                nc.scalar.activation(out=stats[:, 256:512], in_=var[:, :],
                                     func=Act.Abs_reciprocal_sqrt, bias=epst[:, :])
                yield

                # ---- normalize + gelu ----
                bcp = mps.tile([128, 512], f32, tag="m")
                nc.tensor.matmul(out=bcp[:, :], lhsT=ones1_128, rhs=stats[:, :],
                                 start=True, stop=True)
                t1 = spool.tile([128, 256], f32, tag="t1")
                nc.vector.tensor_tensor(out=t1[:, :], in0=accp[:, 0:256],
                                        in1=bcp[:, 0:256], op=Alu.subtract)
                nc.vector.tensor_tensor(out=t1[:, :], in0=t1[:, :],
                                        in1=bcp[:, 256:512], op=Alu.mult)
                nc.vector.tensor_scalar(out=t1[:, :], in0=t1[:, :], scalar1=ln_g,
                                        scalar2=ln_b, op0=Alu.mult, op1=Alu.add)
                gl = spool.tile([128, 256], f32, tag="gl")
                nc.scalar.activation(out=gl[:, :], in_=t1[:, :], func=Act.Gelu)
                yield

                # ---- offsets -> positions -> posT ----
                offp = mps.tile([2, 256], f32, tag="m")
                nc.tensor.matmul(out=offp[:, :], lhsT=pw_wT, rhs=gl[:, :],
                                 start=True, stop=True)
                pos = spool.tile([2, 256], f32, tag="pos")
                nc.vector.tensor_tensor(out=pos[:, :], in0=offp[:, :], in1=ref_yx, op=Alu.add)
                nc.vector.tensor_scalar(out=pos[:, :], in0=pos[:, :], scalar1=1.0,
                                        scalar2=-1.0, op0=Alu.min, op1=Alu.max)
                posT = spool.tile([128, 4], f32, tag="posT")  # (c0y c0x c1y c1x)
                for c in range(2):
                    tp = mps.tile([128, 2], f32, tag="m")
                    nc.tensor.transpose(out=tp[:, :], in_=pos[:, c * 128 : (c + 1) * 128],
                                        identity=eye[0:2, 0:2])
                    nc.vector.tensor_copy(out=posT[:, c * 2 : c * 2 + 2], in_=tp[:, :])
                yield

                # ---- batched index math ----
                pix = spool.tile([128, 4], f32, tag="pix")
                nc.vector.tensor_scalar(out=pix[:, :], in0=posT[:, :], scalar1=1.0,
                                        scalar2=31.5, op0=Alu.add, op1=Alu.mult)
                rnd = spool.tile([128, 4], f32, tag="rnd")
                nc.vector.tensor_scalar(out=rnd[:, :], in0=pix[:, :], scalar1=8388608.0,
                                        scalar2=-8388608.0, op0=Alu.add, op1=Alu.add)
                gt = spool.tile([128, 4], f32, tag="gt")
                nc.vector.tensor_tensor(out=gt[:, :], in0=rnd[:, :], in1=pix[:, :], op=Alu.is_gt)
                p0 = spool.tile([128, 4], f32, tag="p0")
                nc.vector.tensor_tensor(out=p0[:, :], in0=rnd[:, :], in1=gt[:, :], op=Alu.subtract)
                nc.vector.tensor_scalar(out=p0[:, :], in0=p0[:, :], scalar1=62.0,
                                        scalar2=None, op0=Alu.min)
                fr = spool.tile([128, 4], f32, tag="fr")
                nc.vector.tensor_tensor(out=fr[:, :], in0=pix[:, :], in1=p0[:, :], op=Alu.subtract)
                fr1 = spool.tile([128, 4], f32, tag="fr1")
                nc.vector.tensor_scalar(out=fr1[:, :], in0=fr[:, :], scalar1=-1.0,
                                        scalar2=1.0, op0=Alu.mult, op1=Alu.add)
                fc["fr"] = fr
                fc["fr1"] = fr1
                yield

                # chunk views: v=0 -> y, v=1 -> x
                p0v = p0[:, :].rearrange("p (c v) -> p c v", v=2)
                frv = fr[:, :].rearrange("p (c v) -> p c v", v=2)
                fr1v = fr1[:, :].rearrange("p (c v) -> p c v", v=2)

                # ---- bias-window indices (early, enables W prefetch) ----
                q0b = spool.tile([128, 4], f32, tag="q0b")
                nc.vector.tensor_scalar(out=q0b[:, :], in0=p0[:, :], scalar1=-1.0,
                                        scalar2=62.0, op0=Alu.mult, op1=Alu.add)
                q0bv = q0b[:, :].rearrange("p (c v) -> p c v", v=2)
                iw = spool.tile([128, 2], f32, tag="iw")
                nc.vector.scalar_tensor_tensor(out=iw[:, :], in0=q0bv[:, :, 1], scalar=128.0,
                                               in1=q0bv[:, :, 0], op0=Alu.mult, op1=Alu.add)
                nc.vector.tensor_scalar(out=iw[:, :], in0=iw[:, :], scalar1=65.0,
                                        scalar2=float(blk * TBLK), op0=Alu.mult, op1=Alu.add)
                idxw = spool.tile([128, 8], f32, tag="idxw")
                for c in range(2):
                    nc.vector.tensor_tensor(
                        out=idxw[:, c * 4 : (c + 1) * 4],
                        in0=iw[:, c : c + 1].to_broadcast([128, 4]),
                        in1=headoff4, op=Alu.add,
                    )
                idxw_i = spool.tile([128, 8], i32, tag="idxwi")
                nc.vector.tensor_copy(out=idxw_i[:, :], in_=idxw[:, :])
                fc["idxw_i"] = idxw_i
                yield

                # ---- kv bilinear weights + gather indices ----
                wkv = spool.tile([128, 8], f32, tag="wkv")
                wkv4 = wkv[:, :].rearrange("p (c t) -> p c t", t=4)
                nc.vector.tensor_tensor(out=wkv4[:, :, 0], in0=fr1v[:, :, 0], in1=fr1v[:, :, 1], op=Alu.mult)
                nc.vector.tensor_tensor(out=wkv4[:, :, 1], in0=fr1v[:, :, 0], in1=frv[:, :, 1], op=Alu.mult)
                nc.vector.tensor_tensor(out=wkv4[:, :, 2], in0=frv[:, :, 0], in1=fr1v[:, :, 1], op=Alu.mult)
                nc.vector.tensor_tensor(out=wkv4[:, :, 3], in0=frv[:, :, 0], in1=frv[:, :, 1], op=Alu.mult)
                ib = spool.tile([128, 2], f32, tag="ib")
                nc.vector.scalar_tensor_tensor(out=ib[:, :], in0=p0v[:, :, 0], scalar=64.0,
                                               in1=p0v[:, :, 1], op0=Alu.mult, op1=Alu.add)
                idxkv = spool.tile([128, 8], f32, tag="idxkv")
                for c in range(2):
                    nc.vector.tensor_tensor(
                        out=idxkv[:, c * 4 : (c + 1) * 4],
                        in0=ib[:, c : c + 1].to_broadcast([128, 4]),
                        in1=kvoff4, op=Alu.add,
                    )
                idxkv_i = spool.tile([128, 8], i32, tag="idxkvi")
                nc.vector.tensor_copy(out=idxkv_i[:, :], in_=idxkv[:, :])
                G = spool.tile([128, 8, 64], f32, tag="G", bufs=1)
                for j in range(4):
                    nc.gpsimd.indirect_dma_start(
                        out=G[:, j, :], out_offset=None, in_=kvT_ap,
                        in_offset=IndirectOffsetOnAxis(ap=idxkv_i[:, j : j + 1], axis=0),
                    )
                if blk == 0:
                    fc["pend0"] = issue_wpair(idxw_i, 0, 0)
                yield
                for j in range(4, 8):
                    nc.gpsimd.indirect_dma_start(
                        out=G[:, j, :], out_offset=None, in_=kvT_ap,
                        in_offset=IndirectOffsetOnAxis(ap=idxkv_i[:, j : j + 1], axis=0),
                    )
                # diag weight matrices: d0 <- fx, d1 <- 1-fx (per chunk)
                diags = []
                for c in range(2):
                    d0 = spool.tile([128, 128], bf16, tag=f"d0_{c}")
                    d1 = spool.tile([128, 128], bf16, tag=f"d1_{c}")
                    nc.vector.tensor_scalar(out=d0[:, :], in0=eye,
                                            scalar1=fr[:, c * 2 + 1 : c * 2 + 2],
                                            scalar2=None, op0=Alu.mult)
                    nc.vector.tensor_scalar(out=d1[:, :], in0=eye,
                                            scalar1=fr1[:, c * 2 + 1 : c * 2 + 2],
                                            scalar2=None, op0=Alu.mult)
                    diags.append((d0, d1))
                fc["diags"] = diags
                yield

                # ---- gathered kv -> xs -> k (per chunk; chunk 0 unblocks QK) ----
                xs_b = spool.tile([65, 256], bf16, tag="xsb")
                nc.vector.memset(xs_b[64:65, :], 1.0)
                k_b = spool.tile([128, 256], bf16, tag="kb")
                vT1 = spool.tile([128, 256], bf16, tag="vT1")
                for c in range(2):
                    xsT = spool.tile([128, 64], f32, tag="xsT")
                    nc.vector.tensor_scalar(
                        out=xsT[:, :], in0=G[:, c * 4 + 0, :],
                        scalar1=wkv[:, c * 4 : c * 4 + 1], scalar2=None, op0=Alu.mult,
                    )
                    for t in range(1, 4):
                        nc.vector.scalar_tensor_tensor(
                            out=xsT[:, :], in0=G[:, c * 4 + t, :],
                            scalar=wkv[:, c * 4 + t : c * 4 + t + 1], in1=xsT[:, :],
                            op0=Alu.mult, op1=Alu.add,
                        )
                    xsp = mps.tile([64, 128], f32, tag="m")
                    nc.tensor.transpose(out=xsp[:, :], in_=xsT[:, :], identity=eye)
                    nc.scalar.activation(out=xs_b[0:64, c * 128 : (c + 1) * 128],
                                         in_=xsp[:, :], func=Act.Copy)
                    kp = mps.tile([128, 128], f32, tag="m")
                    nc.tensor.matmul(out=kp[:, :],
                                     lhsT=wpb[0:65, blk * 192 : blk * 192 + 128],
                                     rhs=xs_b[:, c * 128 : (c + 1) * 128],
                                     start=True, stop=True)
                    nc.scalar.activation(out=k_b[:, c * 128 : (c + 1) * 128],
                                         in_=kp[:, :], func=Act.Copy)
                    if c == 0:
                        fc["k_b"] = k_b
                        yield
                yield

                # ---- v projection ----
                nc.vector.tensor_copy(out=vT1[:, :], in_=vtm[:, :])
                for c in range(2):
                    vp = mps.tile([128, 64], f32, tag="m")
                    nc.tensor.matmul(
                        out=vp[:, :], lhsT=xs_b[:, c * 128 : (c + 1) * 128],
                        rhs=wpb[0:65, blk * 192 + 128 : blk * 192 + 192],
                        start=True, stop=True,
                    )
                    vv = vT1[:, c * 128 : (c + 1) * 128].rearrange("p (h q) -> p h q", q=32)
                    nc.scalar.activation(
                        out=vv[:, :, 0:16],
                        in_=vp[:, :].rearrange("p (h q) -> p h q", q=16),
                        func=Act.Copy,
                    )
                fc["vT1"] = vT1
                yield

            # ======================= attention =======================
            def attn(blk, fc, R, feeder, out_d=None,
                     pend_in=None, next_fc=None):
                po_wT_sp = cpb[:, 128 + blk * 64 : 128 + (blk + 1) * 64]
                b4 = cpb[:, 0:128]
                po_b_hi = cp[0:64, 524 + blk : 525 + blk]
                q_b = fc["q_b"]
                k_b = fc["k_b"]
                vT1 = fc["vT1"]
                idxw_i = fc["idxw_i"]
                fr1 = fc["fr1"]
                diags = fc["diags"]

                steps = [(h, c) for h in range(4) for c in range(2)]

                def issue_gather(idxsrc, i):
                    h, c = steps[i]
                    return issue_wpair(idxsrc, h, c)

                # ---- sw-pipelined tail, interleaved into the last head ----
                sbps = [None] * 8

                def tail_a(j):
                    sbp = tlps.tile([128, 512], f32, tag="tl")
                    nc.tensor.matmul(out=sbp[:, :], lhsT=b4,
                                     rhs=avs[:, j * 512 : (j + 1) * 512],
                                     start=True, stop=True)
                    sbps[j] = sbp

                def tail_b(j):
                    rcp = spool.tile([128, 512], f32, tag="rcp")
                    act_raw(rcp[:, :], sbps[j][:, :], Act.Reciprocal)
                    on = spool.tile([128, 512], bf16, tag="on")
                    nc.vector.tensor_tensor(out=on[:, :],
                                            in0=avs[:, j * 512 : (j + 1) * 512],
                                            in1=rcp[:, :], op=Alu.mult)
                    op = tlps.tile([128, 512], f32, tag="tl")
                    nc.tensor.matmul(out=op[0:64, :], lhsT=po_wT_sp, rhs=on[:, :],
                                     start=True, stop=True)
                    nc.vector.scalar_tensor_tensor(
                        out=R[0:64, j * 512 : (j + 1) * 512], in0=op[0:64, :],
                        scalar=po_b_hi, in1=R[0:64, j * 512 : (j + 1) * 512],
                        op0=Alu.add, op1=Alu.add,
                    )
                    if out_d is not None:
                        nc.sync.dma_start(
                            out=out_d[64:128, j * 512 : (j + 1) * 512],
                            in_=R[0:64, j * 512 : (j + 1) * 512],
                        )

                avs = apool.tile([128, HWS], bf16, tag="avs")
                pend = dict(pend_in) if pend_in else {}
                if 0 not in pend and "pend0" in fc:
                    pend[0] = fc.pop("pend0")
                for i0 in range(2):
                    if i0 not in pend:
                        pend[i0] = issue_gather(idxw_i, i0)
                pend_next = {}
                P = None
                for i, (h, c) in enumerate(steps):
                    if c == 0:
                        P = ppool.tile([128, 2, HWS], bf16, tag="P")
                    Wt, Dw = pend.pop(i)
                    # y-interp: Y = Wt + (1-fy) * Dw
                    Y = wpool.tile([128, 4160], bf16, tag="Y")
                    nc.vector.tensor_scalar(out=Y[:, :], in0=Dw[:, :],
                                            scalar1=fr1[:, c * 2 : c * 2 + 1],
                                            scalar2=None, op0=Alu.mult)
                    if i + 2 < 8:
                        pend[i + 2] = issue_gather(idxw_i, i + 2)
                    elif i == 7 and next_fc is not None:
                        # prefetch next block's first gather pair
                        while "idxw_i" not in next_fc:
                            if not feeder():
                                break
                        if "idxw_i" in next_fc:
                            pend_next[0] = issue_gather(next_fc["idxw_i"], 0)
                    nc.vector.tensor_tensor(out=Y[:, :], in0=Y[:, :], in1=Wt[:, :], op=Alu.add)
                    Y3 = Y[:, :].rearrange("p (r q) -> p r q", q=65)
                    kh = k_b[h * 32 : h * 32 + 17, c * 128 : (c + 1) * 128]
                    d0, d1 = diags[c]
                    for k in range(4):
                        qk = qkps.tile([128, 1024], f32, tag="qk")
                        for hf in range(2):
                            mc = k * 2 + hf
                            nc.tensor.matmul(
                                out=qk[:, hf * 512 : (hf + 1) * 512], lhsT=kh,
                                rhs=q_b[h * 32 : h * 32 + 17, mc * 512 : (mc + 1) * 512],
                                start=True, stop=False, tile_position=(h * 32, 0),
                            )
                        for hf in range(2):
                            mc = k * 2 + hf
                            nc.tensor.matmul(
                                out=qk[:, hf * 512 : (hf + 1) * 512], lhsT=d0,
                                rhs=Y3[:, mc * 8 : (mc + 1) * 8, 0:64],
                                start=False, stop=False,
                            )
                        for hf in range(2):
                            mc = k * 2 + hf
                            nc.tensor.matmul(
                                out=qk[:, hf * 512 : (hf + 1) * 512], lhsT=d1,
                                rhs=Y3[:, mc * 8 : (mc + 1) * 8, 1:65],
                                start=False, stop=True,
                            )
                        nc.scalar.activation(
                            out=P[:, c, k * 1024 : (k + 1) * 1024], in_=qk[:, :],
                            func=Act.Exp, bias=zb[:, :],
                        )
                    feeder()
                    if c == 1:
                        # AV for this head (+ tail interleaved into head 3)
                        for pr in range(4):
                            a0 = tlps.tile([128, 512], f32, tag="tl")
                            a1 = tlps.tile([128, 512], f32, tag="tl")
                            mca, mcb = pr * 2, pr * 2 + 1
                            for cc in range(2):
                                lw = vT1[:, cc * 128 + h * 32 : cc * 128 + (h + 1) * 32]
                                nc.tensor.matmul(
                                    out=a0[0:32, :], lhsT=lw,
                                    rhs=P[:, cc, mca * 512 : (mca + 1) * 512],
                                    start=(cc == 0), stop=(cc == 1),
                                )
                                nc.tensor.matmul(
                                    out=a1[0:32, :], lhsT=lw,
                                    rhs=P[:, cc, mcb * 512 : (mcb + 1) * 512],
                                    start=(cc == 0), stop=(cc == 1),
                                )
                            nc.vector.tensor_copy(
                                out=avs[h * 32 : (h + 1) * 32, mca * 512 : (mca + 1) * 512],
                                in_=a0[0:32, :])
                            nc.vector.tensor_copy(
                                out=avs[h * 32 : (h + 1) * 32, mcb * 512 : (mcb + 1) * 512],
                                in_=a1[0:32, :])
                            if h == 3:
                                tail_a(pr * 2)
                                tail_a(pr * 2 + 1)
                                if pr >= 1:
                                    tail_b(pr * 2 - 2)
                                    tail_b(pr * 2 - 1)
                        feeder()
                tail_b(6)
                tail_b(7)
                return pend_next

            def make_feeder(gen):
                def feeder():
                    if gen is None:
                        return False
                    try:
                        next(gen)
                        return True
                    except StopIteration:
                        return False
                return feeder

            def drain(gen):
                for _ in gen:
                    pass

            # ======================= schedule =======================
            fc0 = {}
            g0 = front(0, None, kvT0_d, fc0, stream_d=xq1_d)
            next(g0)
            load_bulk()
            drain(g0)
            nc.sync.dma_start(out=o1_d[0:64, :], in_=xi1_d[0:64, :])
            nc.sync.dma_start(out=o2_d[0:64, :], in_=xi2_d[0:64, :])
            fc1 = {}
            g1 = front(1, xq2[0:64, :], kvT0_d, fc1)
            pend1 = attn(0, fc0, xr1, make_feeder(g1), out_d=o1_d,
                         next_fc=fc1)
            drain(g1)
            fc2 = {}
            g2 = front(2, xq2[0:64, :], kvT1_d, fc2)
            pend2 = attn(1, fc1, xr2, make_feeder(g2),
                         pend_in=pend1 or None, next_fc=fc2)
            drain(g2)
            attn(2, fc2, xr2, make_feeder(None), out_d=o2_d,
                 pend_in=pend2 or None)

    nc.compile()
    return nc


def _host_prep(inputs):
    """Build per-core in_maps. inputs: dict of full numpy arrays."""
    import ml_dtypes

    x0, x1, x2 = inputs["x0"], inputs["x1"], inputs["x2"]

    def spread_cols(m):
        # m: [64(in), 64(out)] -> [64(in), 128] with out col h*16+j at h*32+j
        out = np.zeros((m.shape[0], 128), m.dtype)
        for h in range(4):
            out[:, h * 32 : h * 32 + 16] = m[:, h * 16 : (h + 1) * 16]
        return out

    def spread_rows(v):
        # v: [64, k] -> [128, k] with row h*16+j at h*32+j
        out = np.zeros((128,) + v.shape[1:], v.dtype)
        for h in range(4):
            out[h * 32 : h * 32 + 16] = v[h * 16 : (h + 1) * 16]
        return out

    # weight pack bf16: [64, 3*128]  (spread pq_wT)
    wpf = np.zeros((64, 3 * 128), np.float32)
    for b in range(3):
        wpf[:, b * 128 : (b + 1) * 128] = spread_cols(inputs["pq_w"][b].T)
    wpb = np.zeros((65, 3 * 192), ml_dtypes.bfloat16)
    for b in range(3):
        o = b * 192
        pk = np.zeros((65, 128), np.float32)
        pk[0:64] = spread_cols(inputs["pk_w"][b].T * 0.25)
        for h in range(4):
            pk[64, h * 32 : h * 32 + 16] = inputs["pk_b"][b][h * 16 : (h + 1) * 16] * 0.25
        for h in range(4):
            pq_bh = inputs["pq_b"][b][h * 16 : (h + 1) * 16]
            pk[:, h * 32 + 16] = pk[:, h * 32 : h * 32 + 16] @ pq_bh
        wpb[:, o : o + 128] = pk.astype(ml_dtypes.bfloat16)
        wpb[:64, o + 128 : o + 192] = inputs["pv_w"][b].T.astype(ml_dtypes.bfloat16)
        wpb[64, o + 128 : o + 192] = inputs["pv_b"][b].astype(ml_dtypes.bfloat16)
    # const pack [128, 598]
    cp = np.zeros((128, 599), np.float32)
    for h in range(4):
        cp[h * 32 + 16, 598] = 1.0
    cp[:, 0:128] = np.eye(128, dtype=np.float32)
    ys = (np.linspace(0.5, HK - 0.5, HK) / (HK - 1.0)) * 2.0 - 1.0
    cp[0, 128:384] = np.repeat(ys, WK)         # y per n (i-major)
    cp[1, 128:384] = np.tile(ys, HK)           # x per n
    cp[0, 384:512] = 1.0                       # ones1_128
    for h in range(4):
        cp[h * 32 : h * 32 + 16, 520] = 1.0 / 64.0
    for b in range(3):
        cp[:, 521 + b] = spread_rows(inputs["pq_b"][b][:, None])[:, 0]
        cp[0:64, 524 + b] = inputs["po_b"][b]
        bc0 = 527 + b * 21
        cp[:, bc0 : bc0 + 16] = spread_rows(inputs["dw_w"][b].reshape(64, 16))
        dwb_eff = (inputs["dw_b"][b]
                   + inputs["dw_w"][b].reshape(64, 16).sum(1) * inputs["pq_b"][b])
        cp[:, bc0 + 16] = spread_rows(dwb_eff[:, None])[:, 0]
        cp[:, bc0 + 17] = spread_rows(inputs["ln_g"][b][:, None])[:, 0]
        cp[:, bc0 + 18] = spread_rows(inputs["ln_b"][b][:, None])[:, 0]
        cp[:, bc0 + 19 : bc0 + 21] = spread_rows(inputs["pw_w"][b].T)
    cp[:, 590] = 0.0
    cp[:, 591] = 1.0
    cp[:, 592] = 64.0
    cp[:, 593] = 65.0
    for h in range(4):
        cp[:, 594 + h] = float(h * THEAD)
    cpb = np.zeros((128, 320), ml_dtypes.bfloat16)
    b4 = np.zeros((128, 128), np.float32)
    for h in range(4):
        b4[h * 32 + 16, h * 32 : (h + 1) * 32] = 1.0
    cpb[:, 0:128] = b4.astype(ml_dtypes.bfloat16)
    for b in range(3):
        poT = inputs["po_w"][b].T  # [c, o]
        for h in range(4):
            cpb[h * 32 : h * 32 + 16, 128 + b * 64 : 128 + (b + 1) * 64] = poT[
                h * 16 : (h + 1) * 16
            ].astype(ml_dtypes.bfloat16)
    # rpe slice tables bf16: T windows then D (row-diff) windows
    tab = np.zeros((2, NBLK, NH, 64, TROW, TCOL), ml_dtypes.bfloat16)
    rpe = inputs["rpe"]
    for b in range(3):
        for h in range(4):
            pad = np.zeros((129, 128), np.float32)
            pad[0:127, 0:127] = rpe[b, h]
            dif = pad[1:129] - pad[0:128]
            for x0s in range(64):
                tab[0, b, h, x0s] = pad[0:128, x0s : x0s + 65].astype(ml_dtypes.bfloat16)
                tab[1, b, h, x0s] = dif[:, x0s : x0s + 65].astype(ml_dtypes.bfloat16)
    tab = tab.reshape(-1, 1)

    in_maps = []
    for bb in range(B):
        m = {
            "xi1": np.ascontiguousarray(x1[bb].reshape(C, HWS)),
            "xi2": np.ascontiguousarray(x2[bb].reshape(C, HWS)),
            "kvT0": np.ascontiguousarray(x0[bb, :64].reshape(64, HWS).T),
            "xq1": np.ascontiguousarray(x1[bb, :64].reshape(64, HWS)),
            "xq2": np.ascontiguousarray(x2[bb, :64].reshape(64, HWS)),
            "kvT1": np.ascontiguousarray(x1[bb, :64].reshape(64, HWS).T),
            "wpf": wpf,
            "wpb": wpb,
            "cp": cp,
            "cpb": cpb,
            "rpetab": tab,
        }
        in_maps.append(m)
    return in_maps


def kernel(**inputs):
    from concourse.bass_utils import run_bass_kernel_spmd

    if "nc" not in _CACHE:
        _CACHE["nc"] = _build_graph()
    nc = _CACHE["nc"]
    in_maps = _host_prep(inputs)
    res = run_bass_kernel_spmd(nc, in_maps, core_ids=list(range(8)))
    out = np.zeros((NBLK, B, C, H, W), np.float32)
    out[0] = inputs["x0"]
    for bb in range(B):
        out[1, bb] = res.results[bb]["o1"].reshape(C, H, W)
        out[2, bb] = res.results[bb]["o2"].reshape(C, H, W)
    return out


# revision 46
# speedup vs baseline: 1.0953x; 1.0953x over previous
"""Trainium2 Bass kernel for nn_AttentionTD (3-block deformable attention TD).

Self-contained: hardcodes all shapes. Data-parallel over batch B=8 across the
8 NeuronCores; each core runs the full 3-block DAT stack for one batch element.

Pipelined emission: block b+1's front-end (q-proj, offset conv, LN, GELU,
index math, kv gather/proj) is interleaved under block b's attention so the
tensor engine never drains between blocks.
"""

import sys

sys.path.insert(0, "/opt/trn_rl_repo")

import numpy as np

# ---------------- problem constants ----------------
B, C, H, W = 8, 128, 64, 64
NCH = 64          # channels per DAT block
NH, HC = 4, 16    # heads, head channels
KS = 4
HWS = H * W       # 4096
HK = WK = 16
NS = HK * WK      # 256 sample points
EPS = 1e-5
NBLK = 3
# rpe slice table geometry: [blk][h][x0 (64)][row (128)][col (65)]
TROW, TCOL = 128, 65
TSLICE = TROW * TCOL          # 8320
THEAD = 64 * TSLICE           # per (blk,h)
TBLK = NH * THEAD
NTAB = NBLK * TBLK

_CACHE = {}


def _build_graph():
    from concourse import bacc, mybir, tile
    import concourse.bass as bass
    from concourse.bass import IndirectOffsetOnAxis

    f32 = mybir.dt.float32
    bf16 = mybir.dt.bfloat16
    i32 = mybir.dt.int32
    Alu = mybir.AluOpType
    Act = mybir.ActivationFunctionType

    nc = bacc.Bacc("TRN2", target_bir_lowering=False, debug=False, num_devices=8)

    # ---- dram io ----
    xi1_d = nc.dram_tensor("xi1", [C, HWS], f32, kind="ExternalInput").ap()
    xi2_d = nc.dram_tensor("xi2", [C, HWS], f32, kind="ExternalInput").ap()
    kvT0_d = nc.dram_tensor("kvT0", [HWS, NCH], f32, kind="ExternalInput").ap()
    kvT1_d = nc.dram_tensor("kvT1", [HWS, NCH], f32, kind="ExternalInput").ap()
    xq1_d = nc.dram_tensor("xq1", [64, HWS], mybir.dt.float32r, kind="ExternalInput").ap()
    xq2_d = nc.dram_tensor("xq2", [64, HWS], mybir.dt.float32r, kind="ExternalInput").ap()
    wpf_d = nc.dram_tensor("wpf", [64, 3 * 128], mybir.dt.float32r, kind="ExternalInput").ap()
    wpb_d = nc.dram_tensor("wpb", [65, 3 * 192], bf16, kind="ExternalInput").ap()
    cp_d = nc.dram_tensor("cp", [128, 599], f32, kind="ExternalInput").ap()
    cpb_d = nc.dram_tensor("cpb", [128, 320], bf16, kind="ExternalInput").ap()
    tab_d = nc.dram_tensor("rpetab", [2 * NTAB, 1], bf16, kind="ExternalInput").ap()
    o1_d = nc.dram_tensor("o1", [C, HWS], f32, kind="ExternalOutput").ap()
    o2_d = nc.dram_tensor("o2", [C, HWS], f32, kind="ExternalOutput").ap()

    with tile.TileContext(nc) as tc:
        import contextlib

        ctx = contextlib.ExitStack()
        with ctx:
            cpool = ctx.enter_context(tc.tile_pool(name="const", bufs=1))
            xpool = ctx.enter_context(tc.tile_pool(name="xdata", bufs=1))
            qpool = ctx.enter_context(tc.tile_pool(name="qtiles", bufs=2))
            wpool = ctx.enter_context(tc.tile_pool(name="wins", bufs=2))
            ppool = ctx.enter_context(tc.tile_pool(name="probs", bufs=1))
            apool = ctx.enter_context(tc.tile_pool(name="avsp", bufs=1))
            spool = ctx.enter_context(tc.tile_pool(name="small", bufs=2))
            qkps = ctx.enter_context(tc.tile_pool(name="qk", bufs=2, space="PSUM"))
            tlps = ctx.enter_context(tc.tile_pool(name="tl", bufs=3, space="PSUM"))
            mps = ctx.enter_context(tc.tile_pool(name="misc", bufs=1, space="PSUM"))

            # ---- persistent loads ----
            cp = cpool.tile([128, 599], f32, tag="cp")
            nc.sync.dma_start(out=cp[:, :], in_=cp_d)
            wpf = cpool.tile([64, 3 * 128], mybir.dt.float32r, tag="wpf")
            nc.sync.dma_start(out=wpf[:, :], in_=wpf_d)
            wpb = cpool.tile([65, 3 * 192], bf16, tag="wpb")
            nc.sync.dma_start(out=wpb[:, :], in_=wpb_d)
            cpb = cpool.tile([128, 320], bf16, tag="cpb")
            xq2 = xpool.tile([64, HWS], mybir.dt.float32r, tag="xq2")
            xr1 = xpool.tile([64, HWS], f32, tag="xr1")
            xr2 = xpool.tile([64, HWS], f32, tag="xr2")

            def load_bulk():
                nc.sync.dma_start(out=cpb[:, :], in_=cpb_d)
                nc.sync.dma_start(out=xq2[:, :], in_=xq2_d)
                nc.sync.dma_start(out=xr1[:, :], in_=xi1_d[64:128, :])
                nc.sync.dma_start(out=xr2[:, :], in_=xi2_d[64:128, :])


            def act_raw(out, in_, func):
                eng = nc.scalar
                ins = [eng.lower_ap(in_)]
                for v in (0.0, 1.0, 0.0):
                    ins.append(mybir.ImmediateValue(dtype=mybir.dt.float32, value=v))
                return eng.add_instruction(
                    mybir.InstActivation(
                        name=nc.get_next_instruction_name(), func=func,
                        ins=ins, outs=[eng.lower_ap(out)],
                    )
                )

            zb = cpool.tile([128, 1], f32, tag="zb")
            nc.vector.memset(zb[:, :], 0.0)
            epst = cpool.tile([1, 1], f32, tag="epst")
            nc.vector.memset(epst[:, :], EPS)
            # vT1 template: zeros with 1.0 at (c*128 + h*32 + 16)
            vtm = cpool.tile([128, 256], bf16, tag="vtm")
            nc.vector.memset(vtm[:, :], 0.0)
            nc.vector.memset(
                vtm[:, :].rearrange("p (c h q) -> p c h q", c=2, q=32)[:, :, :, 16:17],
                1.0,
            )

            eye = cp[:, 0:128]
            ref_yx = cp[0:2, 128:384]          # row0 = y, row1 = x
            ones1_128 = cp[0:1, 384:512]       # [1,128] ones (bcast lhsT)
            ones128_div = cp[0:128, 520:521]   # 1/64 on data rows, 0 on gaps
            kvoff4 = cp[:, 590:594]            # (0,1,64,65) rows
            headoff4 = cp[:, 594:598]          # (0,T,2T,3T) rows
            ones16 = cp[:, 598:599]            # 1.0 at rows h*32+16


            def issue_wpair(idxsrc, h, c):
                Wt = wpool.tile([128, 4160], bf16, tag="Wt", bufs=3, name="Wt")
                nc.gpsimd.indirect_dma_start(
                    out=Wt[:, :], out_offset=None, in_=tab_d,
                    in_offset=IndirectOffsetOnAxis(
                        ap=idxsrc[:, c * 4 + h : c * 4 + h + 1], axis=0),
                )
                Dw = wpool.tile([128, 4160], bf16, tag="Dw", bufs=3, name="Dw")
                nc.gpsimd.indirect_dma_start(
                    out=Dw[:, :], out_offset=None, in_=tab_d,
                    in_offset=IndirectOffsetOnAxis(
                        ap=idxsrc[:, c * 4 + h : c * 4 + h + 1], axis=0),
                    element_offset=NTAB,
                )
                return Wt, Dw

            # ======================= front-end =======================
            def front(blk, XQb, kvT_ap, fc, stream_d=None):
                bc0 = 527 + blk * 21
                dw_w = cp[:, bc0 : bc0 + 16]
                dw_b = cp[:, bc0 + 16 : bc0 + 17]
                ln_g = cp[:, bc0 + 17 : bc0 + 18]
                ln_b = cp[:, bc0 + 18 : bc0 + 19]
                pw_wT = cp[:, bc0 + 19 : bc0 + 21]
                pq_b_sp = cp[:, 521 + blk : 522 + blk]

                # ---- q projection (f32 copy for conv path + bf16 for QK) ----
                q_f = qpool.tile([128, HWS], mybir.dt.float32r, tag="qf", bufs=1)
                q_b = qpool.tile([128, HWS], bf16, tag="qb")
                f32r = mybir.dt.float32r
                for kq in range(4):
                    if stream_d is not None:
                        xqc = spool.tile([64, 1024], mybir.dt.float32r, tag="xqc", bufs=1)
                        nc.sync.dma_start(out=xqc[:, :],
                                          in_=stream_d[:, kq * 1024 : (kq + 1) * 1024])
                        rhs_kq = xqc[:, :]
                    else:
                        rhs_kq = XQb[:, kq * 1024 : (kq + 1) * 1024]
                    qp = qkps.tile([128, 1024], f32, tag="qk")
                    for hf in range(2):
                        nc.tensor.matmul(
                            out=qp[:, hf * 512 : (hf + 1) * 512],
                            lhsT=wpf[:, blk * 128 : (blk + 1) * 128],
                            rhs=rhs_kq[:, hf * 512 : (hf + 1) * 512],
                            start=True, stop=True,
                        )
                    for hf in range(2):
                        nc.scalar.activation(
                            out=q_f[:, kq * 1024 + hf * 512 : kq * 1024 + (hf + 1) * 512],
                            in_=qp[:, hf * 512 : (hf + 1) * 512], func=Act.Copy)
                    nc.vector.tensor_scalar(
                        out=q_b[:, kq * 1024 : (kq + 1) * 1024], in0=qp[:, :],
                        scalar1=ones16, scalar2=None, op0=Alu.add,
                    )
                    if kq == 1:
                        yield
                fc["q_b"] = q_b
                yield
                yield

                # ---- depthwise 4x4 stride-4 conv (f32r diag matmuls) ----
                q5 = q_f[:, :].rearrange("p (hh a ww b) -> p hh a ww b", hh=16, a=4, ww=16, b=4)
                dwp = mps.tile([128, 256], f32, tag="m")
                for t in range(16):
                    dy, dx = t // 4, t % 4
                    dg = spool.tile([128, 128], mybir.dt.float32r, tag="dg", bufs=2)
                    nc.vector.tensor_scalar(out=dg[:, :], in0=eye,
                                            scalar1=dw_w[:, t : t + 1],
                                            scalar2=None, op0=Alu.mult)
                    nc.tensor.matmul(
                        out=dwp[:, :], lhsT=dg[:, :],
                        rhs=q5[:, :, dy, :, dx],
                        start=(t == 0), stop=(t == 15),
                    )
                    if t == 8:
                        yield
                accp = spool.tile([128, 512], f32, tag="accp", bufs=1)
                acc = accp[:, 0:256]
                nc.vector.tensor_scalar(out=acc, in0=dwp[:, :], scalar1=dw_b,
                                        scalar2=None, op0=Alu.add)
                nc.vector.tensor_tensor(out=accp[:, 256:512], in0=acc,
                                        in1=acc, op=Alu.mult)
                yield

                # ---- layernorm stats ----
                stp = mps.tile([1, 512], f32, tag="m")
                nc.tensor.matmul(out=stp[:, :], lhsT=ones128_div, rhs=accp[:, :],
                                 start=True, stop=True)
                stats = spool.tile([1, 512], f32, tag="stats")
                nc.vector.tensor_copy(out=stats[:, 0:256], in_=stp[:, 0:256])
                var = spool.tile([1, 256], f32, tag="var")
                nc.vector.tensor_tensor(out=var[:, :], in0=stats[:, 0:256],
                                        in1=stats[:, 0:256], op=Alu.mult)
                nc.vector.tensor_tensor(out=var[:, :], in0=stp[:, 256:512],
                                        in1=var[:, :], op=Alu.subtract)
                nc.scalar.activation(out=stats[:, 256:512], in_=var[:, :],
                                     func=Act.Abs_reciprocal_sqrt, bias=epst[:, :])
                yield

                # ---- normalize + gelu ----
                bcp = mps.tile([128, 512], f32, tag="m")
                nc.tensor.matmul(out=bcp[:, :], lhsT=ones1_128, rhs=stats[:, :],
                                 start=True, stop=True)
                t1 = spool.tile([128, 256], f32, tag="t1")
                nc.vector.tensor_tensor(out=t1[:, :], in0=accp[:, 0:256],
                                        in1=bcp[:, 0:256], op=Alu.subtract)
                nc.vector.tensor_tensor(out=t1[:, :], in0=t1[:, :],
                                        in1=bcp[:, 256:512], op=Alu.mult)
                nc.vector.tensor_scalar(out=t1[:, :], in0=t1[:, :], scalar1=ln_g,
                                        scalar2=ln_b, op0=Alu.mult, op1=Alu.add)
                gl = spool.tile([128, 256], f32, tag="gl")
                nc.scalar.activation(out=gl[:, :], in_=t1[:, :], func=Act.Gelu)
                yield

                # ---- offsets -> positions -> posT ----
                offp = mps.tile([2, 256], f32, tag="m")
                nc.tensor.matmul(out=offp[:, :], lhsT=pw_wT, rhs=gl[:, :],
                                 start=True, stop=True)
                pos = spool.tile([2, 256], f32, tag="pos")
                nc.vector.tensor_tensor(out=pos[:, :], in0=offp[:, :], in1=ref_yx, op=Alu.add)
                nc.vector.tensor_scalar(out=pos[:, :], in0=pos[:, :], scalar1=1.0,
                                        scalar2=-1.0, op0=Alu.min, op1=Alu.max)
                posT = spool.tile([128, 4], f32, tag="posT")  # (c0y c0x c1y c1x)
                for c in range(2):
                    tp = mps.tile([128, 2], f32, tag="m")
                    nc.tensor.transpose(out=tp[:, :], in_=pos[:, c * 128 : (c + 1) * 128],
                                        identity=eye[0:2, 0:2])
                    nc.vector.tensor_copy(out=posT[:, c * 2 : c * 2 + 2], in_=tp[:, :])
                yield

                # ---- batched index math ----
                pix = spool.tile([128, 4], f32, tag="pix")
                nc.vector.tensor_scalar(out=pix[:, :], in0=posT[:, :], scalar1=1.0,
                                        scalar2=31.5, op0=Alu.add, op1=Alu.mult)
                rnd = spool.tile([128, 4], f32, tag="rnd")
                nc.vector.tensor_scalar(out=rnd[:, :], in0=pix[:, :], scalar1=8388608.0,
                                        scalar2=-8388608.0, op0=Alu.add, op1=Alu.add)
                gt = spool.tile([128, 4], f32, tag="gt")
                nc.vector.tensor_tensor(out=gt[:, :], in0=rnd[:, :], in1=pix[:, :], op=Alu.is_gt)
                p0 = spool.tile([128, 4], f32, tag="p0")
                nc.vector.tensor_tensor(out=p0[:, :], in0=rnd[:, :], in1=gt[:, :], op=Alu.subtract)
                nc.vector.tensor_scalar(out=p0[:, :], in0=p0[:, :], scalar1=62.0,
                                        scalar2=None, op0=Alu.min)
                fr = spool.tile([128, 4], f32, tag="fr")
                nc.vector.tensor_tensor(out=fr[:, :], in0=pix[:, :], in1=p0[:, :], op=Alu.subtract)
                fr1 = spool.tile([128, 4], f32, tag="fr1")
                nc.vector.tensor_scalar(out=fr1[:, :], in0=fr[:, :], scalar1=-1.0,
                                        scalar2=1.0, op0=Alu.mult, op1=Alu.add)
                fc["fr"] = fr
                fc["fr1"] = fr1
                yield

                # chunk views: v=0 -> y, v=1 -> x
                p0v = p0[:, :].rearrange("p (c v) -> p c v", v=2)
                frv = fr[:, :].rearrange("p (c v) -> p c v", v=2)
                fr1v = fr1[:, :].rearrange("p (c v) -> p c v", v=2)

                # ---- bias-window indices (early, enables W prefetch) ----
                q0b = spool.tile([128, 4], f32, tag="q0b")
                nc.vector.tensor_scalar(out=q0b[:, :], in0=p0[:, :], scalar1=-1.0,
                                        scalar2=62.0, op0=Alu.mult, op1=Alu.add)
                q0bv = q0b[:, :].rearrange("p (c v) -> p c v", v=2)
                iw = spool.tile([128, 2], f32, tag="iw")
                nc.vector.scalar_tensor_tensor(out=iw[:, :], in0=q0bv[:, :, 1], scalar=128.0,
                                               in1=q0bv[:, :, 0], op0=Alu.mult, op1=Alu.add)
                nc.vector.tensor_scalar(out=iw[:, :], in0=iw[:, :], scalar1=65.0,
                                        scalar2=float(blk * TBLK), op0=Alu.mult, op1=Alu.add)
                idxw = spool.tile([128, 8], f32, tag="idxw")
                for c in range(2):
                    nc.vector.tensor_tensor(
                        out=idxw[:, c * 4 : (c + 1) * 4],
                        in0=iw[:, c : c + 1].to_broadcast([128, 4]),
                        in1=headoff4, op=Alu.add,
                    )
                idxw_i = spool.tile([128, 8], i32, tag="idxwi")
                nc.vector.tensor_copy(out=idxw_i[:, :], in_=idxw[:, :])
                fc["idxw_i"] = idxw_i
                yield

                # ---- kv bilinear weights + gather indices ----
                wkv = spool.tile([128, 8], f32, tag="wkv")
                wkv4 = wkv[:, :].rearrange("p (c t) -> p c t", t=4)
                nc.vector.tensor_tensor(out=wkv4[:, :, 0], in0=fr1v[:, :, 0], in1=fr1v[:, :, 1], op=Alu.mult)
                nc.vector.tensor_tensor(out=wkv4[:, :, 1], in0=fr1v[:, :, 0], in1=frv[:, :, 1], op=Alu.mult)
                nc.vector.tensor_tensor(out=wkv4[:, :, 2], in0=frv[:, :, 0], in1=fr1v[:, :, 1], op=Alu.mult)
                nc.vector.tensor_tensor(out=wkv4[:, :, 3], in0=frv[:, :, 0], in1=frv[:, :, 1], op=Alu.mult)
                ib = spool.tile([128, 2], f32, tag="ib")
                nc.vector.scalar_tensor_tensor(out=ib[:, :], in0=p0v[:, :, 0], scalar=64.0,
                                               in1=p0v[:, :, 1], op0=Alu.mult, op1=Alu.add)
                idxkv = spool.tile([128, 8], f32, tag="idxkv")
                for c in range(2):
                    nc.vector.tensor_tensor(
                        out=idxkv[:, c * 4 : (c + 1) * 4],
                        in0=ib[:, c : c + 1].to_broadcast([128, 4]),
                        in1=kvoff4, op=Alu.add,
                    )
                idxkv_i = spool.tile([128, 8], i32, tag="idxkvi")
                nc.vector.tensor_copy(out=idxkv_i[:, :], in_=idxkv[:, :])
                G = spool.tile([128, 8, 64], f32, tag="G", bufs=1)
                for j in range(4):
                    nc.gpsimd.indirect_dma_start(
                        out=G[:, j, :], out_offset=None, in_=kvT_ap,
                        in_offset=IndirectOffsetOnAxis(ap=idxkv_i[:, j : j + 1], axis=0),
                    )
                if blk == 0:
                    fc["pend0"] = issue_wpair(idxw_i, 0, 0)
                yield
                for j in range(4, 8):
                    nc.gpsimd.indirect_dma_start(
                        out=G[:, j, :], out_offset=None, in_=kvT_ap,
                        in_offset=IndirectOffsetOnAxis(ap=idxkv_i[:, j : j + 1], axis=0),
                    )
                # diag weight matrices: d0 <- fx, d1 <- 1-fx (per chunk)
                diags = []
                for c in range(2):
                    d0 = spool.tile([128, 128], bf16, tag=f"d0_{c}")
                    d1 = spool.tile([128, 128], bf16, tag=f"d1_{c}")
                    nc.vector.tensor_scalar(out=d0[:, :], in0=eye,
                                            scalar1=fr[:, c * 2 + 1 : c * 2 + 2],
                                            scalar2=None, op0=Alu.mult)
                    nc.vector.tensor_scalar(out=d1[:, :], in0=eye,
                                            scalar1=fr1[:, c * 2 + 1 : c * 2 + 2],
                                            scalar2=None, op0=Alu.mult)
                    diags.append((d0, d1))
                fc["diags"] = diags
                yield

                # ---- gathered kv -> xs -> k (per chunk; chunk 0 unblocks QK) ----
                xs_b = spool.tile([65, 256], bf16, tag="xsb")
                nc.vector.memset(xs_b[64:65, :], 1.0)
                k_b = spool.tile([128, 256], bf16, tag="kb")
                vT1 = spool.tile([128, 256], bf16, tag="vT1")
                for c in range(2):
                    xsT = spool.tile([128, 64], f32, tag="xsT")
                    nc.vector.tensor_scalar(
                        out=xsT[:, :], in0=G[:, c * 4 + 0, :],
                        scalar1=wkv[:, c * 4 : c * 4 + 1], scalar2=None, op0=Alu.mult,
                    )
                    for t in range(1, 4):
                        nc.vector.scalar_tensor_tensor(
                            out=xsT[:, :], in0=G[:, c * 4 + t, :],
                            scalar=wkv[:, c * 4 + t : c * 4 + t + 1], in1=xsT[:, :],
                            op0=Alu.mult, op1=Alu.add,
                        )
                    xsp = mps.tile([64, 128], f32, tag="m")
                    nc.tensor.transpose(out=xsp[:, :], in_=xsT[:, :], identity=eye)
                    nc.scalar.activation(out=xs_b[0:64, c * 128 : (c + 1) * 128],
                                         in_=xsp[:, :], func=Act.Copy)
                    kp = mps.tile([128, 128], f32, tag="m")
                    nc.tensor.matmul(out=kp[:, :],
                                     lhsT=wpb[0:65, blk * 192 : blk * 192 + 128],
                                     rhs=xs_b[:, c * 128 : (c + 1) * 128],
                                     start=True, stop=True)
                    nc.scalar.activation(out=k_b[:, c * 128 : (c + 1) * 128],
                                         in_=kp[:, :], func=Act.Copy)
                    if c == 0:
                        fc["k_b"] = k_b
                        yield
                yield

                # ---- v projection ----
                nc.vector.tensor_copy(out=vT1[:, :], in_=vtm[:, :])
                for c in range(2):
                    vp = mps.tile([128, 64], f32, tag="m")
                    nc.tensor.matmul(
                        out=vp[:, :], lhsT=xs_b[:, c * 128 : (c + 1) * 128],
                        rhs=wpb[0:65, blk * 192 + 128 : blk * 192 + 192],
                        start=True, stop=True,
                    )
                    vv = vT1[:, c * 128 : (c + 1) * 128].rearrange("p (h q) -> p h q", q=32)
                    nc.scalar.activation(
                        out=vv[:, :, 0:16],
                        in_=vp[:, :].rearrange("p (h q) -> p h q", q=16),
                        func=Act.Copy,
                    )
                fc["vT1"] = vT1
                yield

            # ======================= attention =======================
            def attn(blk, fc, R, feeder, out_d=None,
                     pend_in=None, next_fc=None):
                po_wT_sp = cpb[:, 128 + blk * 64 : 128 + (blk + 1) * 64]
                b4 = cpb[:, 0:128]
                po_b_hi = cp[0:64, 524 + blk : 525 + blk]
                q_b = fc["q_b"]
                k_b = fc["k_b"]
                vT1 = fc["vT1"]
                idxw_i = fc["idxw_i"]
                fr1 = fc["fr1"]
                diags = fc["diags"]

                steps = [(h, c) for h in range(4) for c in range(2)]

                def issue_gather(idxsrc, i):
                    h, c = steps[i]
                    return issue_wpair(idxsrc, h, c)

                # ---- sw-pipelined tail, interleaved into the last head ----
                sbps = [None] * 8

                def tail_a(j):
                    sbp = tlps.tile([128, 512], f32, tag="tl")
                    nc.tensor.matmul(out=sbp[:, :], lhsT=b4,
                                     rhs=avs[:, j * 512 : (j + 1) * 512],
                                     start=True, stop=True)
                    sbps[j] = sbp

                def tail_b(j):
                    rcp = spool.tile([128, 512], f32, tag="rcp")
                    act_raw(rcp[:, :], sbps[j][:, :], Act.Reciprocal)
                    on = spool.tile([128, 512], bf16, tag="on", bufs=1)
                    nc.vector.tensor_tensor(out=on[:, :],
                                            in0=avs[:, j * 512 : (j + 1) * 512],
                                            in1=rcp[:, :], op=Alu.mult)
                    op = tlps.tile([128, 512], f32, tag="tl")
                    nc.tensor.matmul(out=op[0:64, :], lhsT=po_wT_sp, rhs=on[:, :],
                                     start=True, stop=True)
                    nc.vector.scalar_tensor_tensor(
                        out=R[0:64, j * 512 : (j + 1) * 512], in0=op[0:64, :],
                        scalar=po_b_hi, in1=R[0:64, j * 512 : (j + 1) * 512],
                        op0=Alu.add, op1=Alu.add,
                    )
                    if out_d is not None:
                        nc.sync.dma_start(
                            out=out_d[64:128, j * 512 : (j + 1) * 512],
                            in_=R[0:64, j * 512 : (j + 1) * 512],
                        )

                avs = apool.tile([128, HWS], bf16, tag="avs")
                pend = dict(pend_in) if pend_in else {}
                if 0 not in pend and "pend0" in fc:
                    pend[0] = fc.pop("pend0")
                for i0 in range(2):
                    if i0 not in pend:
                        pend[i0] = issue_gather(idxw_i, i0)
                pend_next = {}
                P = None
                for i, (h, c) in enumerate(steps):
                    if c == 0:
                        P = ppool.tile([128, 2, HWS], bf16, tag="P")
                    Wt, Dw = pend.pop(i)
                    # y-interp: Y = Wt + (1-fy) * Dw
                    Y = wpool.tile([128, 4160], bf16, tag="Y")
                    nc.vector.tensor_scalar(out=Y[:, :], in0=Dw[:, :],
                                            scalar1=fr1[:, c * 2 : c * 2 + 1],
                                            scalar2=None, op0=Alu.mult)
                    if i + 2 < 8:
                        pend[i + 2] = issue_gather(idxw_i, i + 2)
                    elif next_fc is not None:
                        # prefetch next block's first gather pairs
                        while "idxw_i" not in next_fc:
                            if not feeder():
                                break
                        if "idxw_i" in next_fc:
                            pend_next[i - 6] = issue_gather(next_fc["idxw_i"], i - 6)
                    nc.vector.tensor_tensor(out=Y[:, :], in0=Y[:, :], in1=Wt[:, :], op=Alu.add)
                    Y3 = Y[:, :].rearrange("p (r q) -> p r q", q=65)
                    kh = k_b[h * 32 : h * 32 + 17, c * 128 : (c + 1) * 128]
                    d0, d1 = diags[c]
                    for k in range(4):
                        qk = qkps.tile([128, 1024], f32, tag="qk")
                        for hf in range(2):
                            mc = k * 2 + hf
                            nc.tensor.matmul(
                                out=qk[:, hf * 512 : (hf + 1) * 512], lhsT=kh,
                                rhs=q_b[h * 32 : h * 32 + 17, mc * 512 : (mc + 1) * 512],
                                start=True, stop=False, tile_position=(h * 32, 0),
                            )
                        for hf in range(2):
                            mc = k * 2 + hf
                            nc.tensor.matmul(
                                out=qk[:, hf * 512 : (hf + 1) * 512], lhsT=d0,
                                rhs=Y3[:, mc * 8 : (mc + 1) * 8, 0:64],
                                start=False, stop=False,
                            )
                        for hf in range(2):
                            mc = k * 2 + hf
                            nc.tensor.matmul(
                                out=qk[:, hf * 512 : (hf + 1) * 512], lhsT=d1,
                                rhs=Y3[:, mc * 8 : (mc + 1) * 8, 1:65],
                                start=False, stop=True,
                            )
                        nc.scalar.activation(
                            out=P[:, c, k * 1024 : (k + 1) * 1024], in_=qk[:, :],
                            func=Act.Exp, bias=zb[:, :],
                        )
                    feeder()
                    if c == 1:
                        # AV for this head (+ tail interleaved into head 3)
                        for pr in range(4):
                            a0 = tlps.tile([128, 512], f32, tag="tl")
                            a1 = tlps.tile([128, 512], f32, tag="tl")
                            mca, mcb = pr * 2, pr * 2 + 1
                            for cc in range(2):
                                lw = vT1[:, cc * 128 + h * 32 : cc * 128 + (h + 1) * 32]
                                nc.tensor.matmul(
                                    out=a0[0:32, :], lhsT=lw,
                                    rhs=P[:, cc, mca * 512 : (mca + 1) * 512],
                                    start=(cc == 0), stop=(cc == 1),
                                )
                                nc.tensor.matmul(
                                    out=a1[0:32, :], lhsT=lw,
                                    rhs=P[:, cc, mcb * 512 : (mcb + 1) * 512],
                                    start=(cc == 0), stop=(cc == 1),
                                )
                            nc.vector.tensor_copy(
                                out=avs[h * 32 : (h + 1) * 32, mca * 512 : (mca + 1) * 512],
                                in_=a0[0:32, :])
                            nc.vector.tensor_copy(
                                out=avs[h * 32 : (h + 1) * 32, mcb * 512 : (mcb + 1) * 512],
                                in_=a1[0:32, :])
                            if h == 3:
                                tail_a(pr * 2)
                                tail_a(pr * 2 + 1)
                                if pr >= 1:
                                    tail_b(pr * 2 - 2)
                                    tail_b(pr * 2 - 1)
                        feeder()
                tail_b(6)
                tail_b(7)
                return pend_next

            def make_feeder(gen):
                def feeder():
                    if gen is None:
                        return False
                    try:
                        next(gen)
                        return True
                    except StopIteration:
                        return False
                return feeder

            def drain(gen):
                for _ in gen:
                    pass

            # ======================= schedule =======================
            fc0 = {}
            g0 = front(0, None, kvT0_d, fc0, stream_d=xq1_d)
            next(g0)
            load_bulk()
            drain(g0)
            nc.sync.dma_start(out=o1_d[0:64, :], in_=xi1_d[0:64, :])
            nc.sync.dma_start(out=o2_d[0:64, :], in_=xi2_d[0:64, :])
            fc1 = {}
            g1 = front(1, xq2[0:64, :], kvT0_d, fc1)
            pend1 = attn(0, fc0, xr1, make_feeder(g1), out_d=o1_d,
                         next_fc=fc1)
            drain(g1)
            fc2 = {}
            g2 = front(2, xq2[0:64, :], kvT1_d, fc2)
            pend2 = attn(1, fc1, xr2, make_feeder(g2),
                         pend_in=pend1 or None, next_fc=fc2)
            drain(g2)
            attn(2, fc2, xr2, make_feeder(None), out_d=o2_d,
                 pend_in=pend2 or None)

    nc.compile()
    return nc


def _host_prep(inputs):
    """Build per-core in_maps. inputs: dict of full numpy arrays."""
    import ml_dtypes

    x0, x1, x2 = inputs["x0"], inputs["x1"], inputs["x2"]

    def spread_cols(m):
        # m: [64(in), 64(out)] -> [64(in), 128] with out col h*16+j at h*32+j
        out = np.zeros((m.shape[0], 128), m.dtype)
        for h in range(4):
            out[:, h * 32 : h * 32 + 16] = m[:, h * 16 : (h + 1) * 16]
        return out

    def spread_rows(v):
        # v: [64, k] -> [128, k] with row h*16+j at h*32+j
        out = np.zeros((128,) + v.shape[1:], v.dtype)
        for h in range(4):
            out[h * 32 : h * 32 + 16] = v[h * 16 : (h + 1) * 16]
        return out

    # weight pack bf16: [64, 3*128]  (spread pq_wT)
    wpf = np.zeros((64, 3 * 128), np.float32)
    for b in range(3):
        wpf[:, b * 128 : (b + 1) * 128] = spread_cols(inputs["pq_w"][b].T)
    wpb = np.zeros((65, 3 * 192), ml_dtypes.bfloat16)
    for b in range(3):
        o = b * 192
        pk = np.zeros((65, 128), np.float32)
        pk[0:64] = spread_cols(inputs["pk_w"][b].T * 0.25)
        for h in range(4):
            pk[64, h * 32 : h * 32 + 16] = inputs["pk_b"][b][h * 16 : (h + 1) * 16] * 0.25
        for h in range(4):
            pq_bh = inputs["pq_b"][b][h * 16 : (h + 1) * 16]
            pk[:, h * 32 + 16] = pk[:, h * 32 : h * 32 + 16] @ pq_bh
        wpb[:, o : o + 128] = pk.astype(ml_dtypes.bfloat16)
        wpb[:64, o + 128 : o + 192] = inputs["pv_w"][b].T.astype(ml_dtypes.bfloat16)
        wpb[64, o + 128 : o + 192] = inputs["pv_b"][b].astype(ml_dtypes.bfloat16)
    # const pack [128, 598]
    cp = np.zeros((128, 599), np.float32)
    for h in range(4):
        cp[h * 32 + 16, 598] = 1.0
    cp[:, 0:128] = np.eye(128, dtype=np.float32)
    ys = (np.linspace(0.5, HK - 0.5, HK) / (HK - 1.0)) * 2.0 - 1.0
    cp[0, 128:384] = np.repeat(ys, WK)         # y per n (i-major)
    cp[1, 128:384] = np.tile(ys, HK)           # x per n
    cp[0, 384:512] = 1.0                       # ones1_128
    for h in range(4):
        cp[h * 32 : h * 32 + 16, 520] = 1.0 / 64.0
    for b in range(3):
        cp[:, 521 + b] = spread_rows(inputs["pq_b"][b][:, None])[:, 0]
        cp[0:64, 524 + b] = inputs["po_b"][b]
        bc0 = 527 + b * 21
        cp[:, bc0 : bc0 + 16] = spread_rows(inputs["dw_w"][b].reshape(64, 16))
        dwb_eff = (inputs["dw_b"][b]
                   + inputs["dw_w"][b].reshape(64, 16).sum(1) * inputs["pq_b"][b])
        cp[:, bc0 + 16] = spread_rows(dwb_eff[:, None])[:, 0]
        cp[:, bc0 + 17] = spread_rows(inputs["ln_g"][b][:, None])[:, 0]
        cp[:, bc0 + 18] = spread_rows(inputs["ln_b"][b][:, None])[:, 0]
        cp[:, bc0 + 19 : bc0 + 21] = spread_rows(inputs["pw_w"][b].T)
    cp[:, 590] = 0.0
    cp[:, 591] = 1.0
    cp[:, 592] = 64.0
    cp[:, 593] = 65.0
    for h in range(4):
        cp[:, 594 + h] = float(h * THEAD)
    cpb = np.zeros((128, 320), ml_dtypes.bfloat16)
    b4 = np.zeros((128, 128), np.float32)
    for h in range(4):
        b4[h * 32 + 16, h * 32 : (h + 1) * 32] = 1.0
    cpb[:, 0:128] = b4.astype(ml_dtypes.bfloat16)
    for b in range(3):
        poT = inputs["po_w"][b].T  # [c, o]
        for h in range(4):
            cpb[h * 32 : h * 32 + 16, 128 + b * 64 : 128 + (b + 1) * 64] = poT[
                h * 16 : (h + 1) * 16
            ].astype(ml_dtypes.bfloat16)
    # rpe slice tables bf16: T windows then D (row-diff) windows
    tab = np.zeros((2, NBLK, NH, 64, TROW, TCOL), ml_dtypes.bfloat16)
    rpe = inputs["rpe"]
    for b in range(3):
        for h in range(4):
            pad = np.zeros((129, 128), np.float32)
            pad[0:127, 0:127] = rpe[b, h]
            dif = pad[1:129] - pad[0:128]
            for x0s in range(64):
                tab[0, b, h, x0s] = pad[0:128, x0s : x0s + 65].astype(ml_dtypes.bfloat16)
                tab[1, b, h, x0s] = dif[:, x0s : x0s + 65].astype(ml_dtypes.bfloat16)
    tab = tab.reshape(-1, 1)

    in_maps = []
    for bb in range(B):
        m = {
            "xi1": np.ascontiguousarray(x1[bb].reshape(C, HWS)),
            "xi2": np.ascontiguousarray(x2[bb].reshape(C, HWS)),
            "kvT0": np.ascontiguousarray(x0[bb, :64].reshape(64, HWS).T),
            "xq1": np.ascontiguousarray(x1[bb, :64].reshape(64, HWS)),
            "xq2": np.ascontiguousarray(x2[bb, :64].reshape(64, HWS)),
            "kvT1": np.ascontiguousarray(x1[bb, :64].reshape(64, HWS).T),
            "wpf": wpf,
            "wpb": wpb,
            "cp": cp,
            "cpb": cpb,
            "rpetab": tab,
        }
        in_maps.append(m)
    return in_maps


def kernel(**inputs):
    from concourse.bass_utils import run_bass_kernel_spmd

    if "nc" not in _CACHE:
        _CACHE["nc"] = _build_graph()
    nc = _CACHE["nc"]
    in_maps = _host_prep(inputs)
    res = run_bass_kernel_spmd(nc, in_maps, core_ids=list(range(8)))
    out = np.zeros((NBLK, B, C, H, W), np.float32)
    out[0] = inputs["x0"]
    for bb in range(B):
        out[1, bb] = res.results[bb]["o1"].reshape(C, H, W)
        out[2, bb] = res.results[bb]["o2"].reshape(C, H, W)
    return out


# revision 47
# speedup vs baseline: 1.0975x; 1.0020x over previous
"""Trainium2 Bass kernel for nn_AttentionTD (3-block deformable attention TD).

Self-contained: hardcodes all shapes. Data-parallel over batch B=8 across the
8 NeuronCores; each core runs the full 3-block DAT stack for one batch element.

Pipelined emission: block b+1's front-end (q-proj, offset conv, LN, GELU,
index math, kv gather/proj) is interleaved under block b's attention so the
tensor engine never drains between blocks.
"""

import sys

sys.path.insert(0, "/opt/trn_rl_repo")

import numpy as np

# ---------------- problem constants ----------------
B, C, H, W = 8, 128, 64, 64
NCH = 64          # channels per DAT block
NH, HC = 4, 16    # heads, head channels
KS = 4
HWS = H * W       # 4096
HK = WK = 16
NS = HK * WK      # 256 sample points
EPS = 1e-5
NBLK = 3
# rpe slice table geometry: [blk][h][x0 (64)][row (128)][col (65)]
TROW, TCOL = 128, 65
TSLICE = TROW * TCOL          # 8320
THEAD = 64 * TSLICE           # per (blk,h)
TBLK = NH * THEAD
NTAB = NBLK * TBLK

_CACHE = {}


def _build_graph():
    from concourse import bacc, mybir, tile
    import concourse.bass as bass
    from concourse.bass import IndirectOffsetOnAxis

    f32 = mybir.dt.float32
    bf16 = mybir.dt.bfloat16
    i32 = mybir.dt.int32
    Alu = mybir.AluOpType
    Act = mybir.ActivationFunctionType

    nc = bacc.Bacc("TRN2", target_bir_lowering=False, debug=False, num_devices=8)

    # ---- dram io ----
    xi1_d = nc.dram_tensor("xi1", [C, HWS], f32, kind="ExternalInput").ap()
    xi2_d = nc.dram_tensor("xi2", [C, HWS], f32, kind="ExternalInput").ap()
    kvT0_d = nc.dram_tensor("kvT0", [HWS, NCH], f32, kind="ExternalInput").ap()
    kvT1_d = nc.dram_tensor("kvT1", [HWS, NCH], f32, kind="ExternalInput").ap()
    xq1_d = nc.dram_tensor("xq1", [64, HWS], mybir.dt.float32r, kind="ExternalInput").ap()
    xq2_d = nc.dram_tensor("xq2", [64, HWS], mybir.dt.float32r, kind="ExternalInput").ap()
    wpf_d = nc.dram_tensor("wpf", [64, 3 * 128], mybir.dt.float32r, kind="ExternalInput").ap()
    wpb_d = nc.dram_tensor("wpb", [65, 3 * 192], bf16, kind="ExternalInput").ap()
    cp_d = nc.dram_tensor("cp", [128, 599], f32, kind="ExternalInput").ap()
    cpb_d = nc.dram_tensor("cpb", [128, 320], bf16, kind="ExternalInput").ap()
    tab_d = nc.dram_tensor("rpetab", [2 * NTAB, 1], bf16, kind="ExternalInput").ap()
    o1_d = nc.dram_tensor("o1", [C, HWS], f32, kind="ExternalOutput").ap()
    o2_d = nc.dram_tensor("o2", [C, HWS], f32, kind="ExternalOutput").ap()

    with tile.TileContext(nc) as tc:
        import contextlib

        ctx = contextlib.ExitStack()
        with ctx:
            cpool = ctx.enter_context(tc.tile_pool(name="const", bufs=1))
            xpool = ctx.enter_context(tc.tile_pool(name="xdata", bufs=1))
            qpool = ctx.enter_context(tc.tile_pool(name="qtiles", bufs=2))
            wpool = ctx.enter_context(tc.tile_pool(name="wins", bufs=2))
            ppool = ctx.enter_context(tc.tile_pool(name="probs", bufs=1))
            apool = ctx.enter_context(tc.tile_pool(name="avsp", bufs=1))
            spool = ctx.enter_context(tc.tile_pool(name="small", bufs=2))
            qkps = ctx.enter_context(tc.tile_pool(name="qk", bufs=2, space="PSUM"))
            tlps = ctx.enter_context(tc.tile_pool(name="tl", bufs=3, space="PSUM"))
            mps = ctx.enter_context(tc.tile_pool(name="misc", bufs=1, space="PSUM"))

            # ---- persistent loads ----
            cp = cpool.tile([128, 599], f32, tag="cp")
            nc.sync.dma_start(out=cp[:, :], in_=cp_d)
            wpf = cpool.tile([64, 3 * 128], mybir.dt.float32r, tag="wpf")
            nc.sync.dma_start(out=wpf[:, :], in_=wpf_d)
            wpb = cpool.tile([65, 3 * 192], bf16, tag="wpb")
            nc.sync.dma_start(out=wpb[:, :], in_=wpb_d)
            cpb = cpool.tile([128, 320], bf16, tag="cpb")
            xq2 = xpool.tile([64, HWS], mybir.dt.float32r, tag="xq2")
            xr1 = xpool.tile([64, HWS], f32, tag="xr1")
            xr2 = xpool.tile([64, HWS], f32, tag="xr2")

            def load_bulk():
                nc.sync.dma_start(out=cpb[:, :], in_=cpb_d)
                nc.sync.dma_start(out=xq2[:, :], in_=xq2_d)
                nc.sync.dma_start(out=xr1[:, :], in_=xi1_d[64:128, :])
                nc.sync.dma_start(out=xr2[:, :], in_=xi2_d[64:128, :])


            def act_raw(out, in_, func):
                eng = nc.scalar
                ins = [eng.lower_ap(in_)]
                for v in (0.0, 1.0, 0.0):
                    ins.append(mybir.ImmediateValue(dtype=mybir.dt.float32, value=v))
                return eng.add_instruction(
                    mybir.InstActivation(
                        name=nc.get_next_instruction_name(), func=func,
                        ins=ins, outs=[eng.lower_ap(out)],
                    )
                )

            zb = cpool.tile([128, 1], f32, tag="zb")
            nc.vector.memset(zb[:, :], 0.0)
            epst = cpool.tile([1, 1], f32, tag="epst")
            nc.vector.memset(epst[:, :], EPS)
            # vT1 template: zeros with 1.0 at (c*128 + h*32 + 16)
            vtm = cpool.tile([128, 256], bf16, tag="vtm")
            nc.vector.memset(vtm[:, :], 0.0)
            nc.vector.memset(
                vtm[:, :].rearrange("p (c h q) -> p c h q", c=2, q=32)[:, :, :, 16:17],
                1.0,
            )

            eye = cp[:, 0:128]
            ref_yx = cp[0:2, 128:384]          # row0 = y, row1 = x
            ones1_128 = cp[0:1, 384:512]       # [1,128] ones (bcast lhsT)
            ones128_div = cp[0:128, 520:521]   # 1/64 on data rows, 0 on gaps
            kvoff4 = cp[:, 590:594]            # (0,1,64,65) rows
            headoff4 = cp[:, 594:598]          # (0,T,2T,3T) rows
            ones16 = cp[:, 598:599]            # 1.0 at rows h*32+16


            def issue_wpair(idxsrc, h, c):
                Wt = wpool.tile([128, 4160], bf16, tag="Wt", bufs=3, name="Wt")
                nc.gpsimd.indirect_dma_start(
                    out=Wt[:, :], out_offset=None, in_=tab_d,
                    in_offset=IndirectOffsetOnAxis(
                        ap=idxsrc[:, c * 4 + h : c * 4 + h + 1], axis=0),
                )
                Dw = wpool.tile([128, 4160], bf16, tag="Dw", bufs=3, name="Dw")
                nc.gpsimd.indirect_dma_start(
                    out=Dw[:, :], out_offset=None, in_=tab_d,
                    in_offset=IndirectOffsetOnAxis(
                        ap=idxsrc[:, c * 4 + h : c * 4 + h + 1], axis=0),
                    element_offset=NTAB,
                )
                return Wt, Dw

            # ======================= front-end =======================
            def front(blk, XQb, kvT_ap, fc, stream_d=None):
                bc0 = 527 + blk * 21
                dw_w = cp[:, bc0 : bc0 + 16]
                dw_b = cp[:, bc0 + 16 : bc0 + 17]
                ln_g = cp[:, bc0 + 17 : bc0 + 18]
                ln_b = cp[:, bc0 + 18 : bc0 + 19]
                pw_wT = cp[:, bc0 + 19 : bc0 + 21]
                pq_b_sp = cp[:, 521 + blk : 522 + blk]

                # ---- q projection (f32 copy for conv path + bf16 for QK) ----
                q_f = qpool.tile([128, HWS], mybir.dt.float32r, tag="qf", bufs=1)
                q_b = qpool.tile([128, HWS], bf16, tag="qb")
                f32r = mybir.dt.float32r
                for kq in range(4):
                    if stream_d is not None:
                        xqc = spool.tile([64, 1024], mybir.dt.float32r, tag="xqc", bufs=1)
                        nc.sync.dma_start(out=xqc[:, :],
                                          in_=stream_d[:, kq * 1024 : (kq + 1) * 1024])
                        rhs_kq = xqc[:, :]
                    else:
                        rhs_kq = XQb[:, kq * 1024 : (kq + 1) * 1024]
                    qp = qkps.tile([128, 1024], f32, tag="qk")
                    for hf in range(2):
                        nc.tensor.matmul(
                            out=qp[:, hf * 512 : (hf + 1) * 512],
                            lhsT=wpf[:, blk * 128 : (blk + 1) * 128],
                            rhs=rhs_kq[:, hf * 512 : (hf + 1) * 512],
                            start=True, stop=True,
                        )
                    for hf in range(2):
                        nc.scalar.activation(
                            out=q_f[:, kq * 1024 + hf * 512 : kq * 1024 + (hf + 1) * 512],
                            in_=qp[:, hf * 512 : (hf + 1) * 512], func=Act.Copy)
                    nc.vector.tensor_scalar(
                        out=q_b[:, kq * 1024 : (kq + 1) * 1024], in0=qp[:, :],
                        scalar1=ones16, scalar2=None, op0=Alu.add,
                    )
                    if kq == 1:
                        yield
                fc["q_b"] = q_b
                yield
                yield

                # ---- depthwise 4x4 stride-4 conv (f32r diag matmuls) ----
                q5 = q_f[:, :].rearrange("p (hh a ww b) -> p hh a ww b", hh=16, a=4, ww=16, b=4)
                dwp = mps.tile([128, 256], f32, tag="m")
                for t in range(16):
                    dy, dx = t // 4, t % 4
                    dg = spool.tile([128, 128], mybir.dt.float32r, tag="dg", bufs=2)
                    nc.vector.tensor_scalar(out=dg[:, :], in0=eye,
                                            scalar1=dw_w[:, t : t + 1],
                                            scalar2=None, op0=Alu.mult)
                    nc.tensor.matmul(
                        out=dwp[:, :], lhsT=dg[:, :],
                        rhs=q5[:, :, dy, :, dx],
                        start=(t == 0), stop=(t == 15),
                    )
                    if t == 8:
                        yield
                accp = spool.tile([128, 512], f32, tag="accp", bufs=1)
                acc = accp[:, 0:256]
                nc.vector.tensor_scalar(out=acc, in0=dwp[:, :], scalar1=dw_b,
                                        scalar2=None, op0=Alu.add)
                nc.vector.tensor_tensor(out=accp[:, 256:512], in0=acc,
                                        in1=acc, op=Alu.mult)
                yield

                # ---- layernorm stats ----
                stp = mps.tile([1, 512], f32, tag="m")
                nc.tensor.matmul(out=stp[:, :], lhsT=ones128_div, rhs=accp[:, :],
                                 start=True, stop=True)
                stats = spool.tile([1, 512], f32, tag="stats")
                nc.vector.tensor_copy(out=stats[:, 0:256], in_=stp[:, 0:256])
                var = spool.tile([1, 256], f32, tag="var")
                nc.vector.tensor_tensor(out=var[:, :], in0=stats[:, 0:256],
                                        in1=stats[:, 0:256], op=Alu.mult)
                nc.vector.tensor_tensor(out=var[:, :], in0=stp[:, 256:512],
                                        in1=var[:, :], op=Alu.subtract)
                nc.scalar.activation(out=stats[:, 256:512], in_=var[:, :],
                                     func=Act.Abs_reciprocal_sqrt, bias=epst[:, :])
                yield

                # ---- normalize + gelu ----
                bcp = mps.tile([128, 512], f32, tag="m")
                nc.tensor.matmul(out=bcp[:, :], lhsT=ones1_128, rhs=stats[:, :],
                                 start=True, stop=True)
                t1 = spool.tile([128, 256], f32, tag="t1")
                nc.vector.tensor_tensor(out=t1[:, :], in0=accp[:, 0:256],
                                        in1=bcp[:, 0:256], op=Alu.subtract)
                nc.vector.tensor_tensor(out=t1[:, :], in0=t1[:, :],
                                        in1=bcp[:, 256:512], op=Alu.mult)
                nc.vector.tensor_scalar(out=t1[:, :], in0=t1[:, :], scalar1=ln_g,
                                        scalar2=ln_b, op0=Alu.mult, op1=Alu.add)
                gl = spool.tile([128, 256], f32, tag="gl")
                nc.scalar.activation(out=gl[:, :], in_=t1[:, :], func=Act.Gelu)
                yield

                # ---- offsets -> positions -> posT ----
                offp = mps.tile([2, 256], f32, tag="m")
                nc.tensor.matmul(out=offp[:, :], lhsT=pw_wT, rhs=gl[:, :],
                                 start=True, stop=True)
                pos = spool.tile([2, 256], f32, tag="pos")
                nc.vector.tensor_tensor(out=pos[:, :], in0=offp[:, :], in1=ref_yx, op=Alu.add)
                nc.vector.tensor_scalar(out=pos[:, :], in0=pos[:, :], scalar1=1.0,
                                        scalar2=-1.0, op0=Alu.min, op1=Alu.max)
                posT = spool.tile([128, 4], f32, tag="posT")  # (c0y c0x c1y c1x)
                for c in range(2):
                    tp = mps.tile([128, 2], f32, tag="m")
                    nc.tensor.transpose(out=tp[:, :], in_=pos[:, c * 128 : (c + 1) * 128],
                                        identity=eye[0:2, 0:2])
                    nc.vector.tensor_copy(out=posT[:, c * 2 : c * 2 + 2], in_=tp[:, :])
                yield

                # ---- batched index math ----
                pix = spool.tile([128, 4], f32, tag="pix")
                nc.vector.tensor_scalar(out=pix[:, :], in0=posT[:, :], scalar1=1.0,
                                        scalar2=31.5, op0=Alu.add, op1=Alu.mult)
                rnd = spool.tile([128, 4], f32, tag="rnd")
                nc.vector.tensor_scalar(out=rnd[:, :], in0=pix[:, :], scalar1=8388608.0,
                                        scalar2=-8388608.0, op0=Alu.add, op1=Alu.add)
                gt = spool.tile([128, 4], f32, tag="gt")
                nc.vector.tensor_tensor(out=gt[:, :], in0=rnd[:, :], in1=pix[:, :], op=Alu.is_gt)
                p0 = spool.tile([128, 4], f32, tag="p0")
                nc.vector.tensor_tensor(out=p0[:, :], in0=rnd[:, :], in1=gt[:, :], op=Alu.subtract)
                nc.vector.tensor_scalar(out=p0[:, :], in0=p0[:, :], scalar1=62.0,
                                        scalar2=None, op0=Alu.min)
                fr = spool.tile([128, 4], f32, tag="fr")
                nc.vector.tensor_tensor(out=fr[:, :], in0=pix[:, :], in1=p0[:, :], op=Alu.subtract)
                fr1 = spool.tile([128, 4], f32, tag="fr1")
                nc.vector.tensor_scalar(out=fr1[:, :], in0=fr[:, :], scalar1=-1.0,
                                        scalar2=1.0, op0=Alu.mult, op1=Alu.add)
                fc["fr"] = fr
                fc["fr1"] = fr1
                yield

                # chunk views: v=0 -> y, v=1 -> x
                p0v = p0[:, :].rearrange("p (c v) -> p c v", v=2)
                frv = fr[:, :].rearrange("p (c v) -> p c v", v=2)
                fr1v = fr1[:, :].rearrange("p (c v) -> p c v", v=2)

                # ---- bias-window indices (early, enables W prefetch) ----
                q0b = spool.tile([128, 4], f32, tag="q0b")
                nc.vector.tensor_scalar(out=q0b[:, :], in0=p0[:, :], scalar1=-1.0,
                                        scalar2=62.0, op0=Alu.mult, op1=Alu.add)
                q0bv = q0b[:, :].rearrange("p (c v) -> p c v", v=2)
                iw = spool.tile([128, 2], f32, tag="iw")
                nc.vector.scalar_tensor_tensor(out=iw[:, :], in0=q0bv[:, :, 1], scalar=128.0,
                                               in1=q0bv[:, :, 0], op0=Alu.mult, op1=Alu.add)
                nc.vector.tensor_scalar(out=iw[:, :], in0=iw[:, :], scalar1=65.0,
                                        scalar2=float(blk * TBLK), op0=Alu.mult, op1=Alu.add)
                idxw = spool.tile([128, 8], f32, tag="idxw")
                for c in range(2):
                    nc.vector.tensor_tensor(
                        out=idxw[:, c * 4 : (c + 1) * 4],
                        in0=iw[:, c : c + 1].to_broadcast([128, 4]),
                        in1=headoff4, op=Alu.add,
                    )
                idxw_i = spool.tile([128, 8], i32, tag="idxwi")
                nc.vector.tensor_copy(out=idxw_i[:, :], in_=idxw[:, :])
                fc["idxw_i"] = idxw_i
                yield

                # ---- kv bilinear weights + gather indices ----
                wkv = spool.tile([128, 8], f32, tag="wkv")
                wkv4 = wkv[:, :].rearrange("p (c t) -> p c t", t=4)
                nc.vector.tensor_tensor(out=wkv4[:, :, 0], in0=fr1v[:, :, 0], in1=fr1v[:, :, 1], op=Alu.mult)
                nc.vector.tensor_tensor(out=wkv4[:, :, 1], in0=fr1v[:, :, 0], in1=frv[:, :, 1], op=Alu.mult)
                nc.vector.tensor_tensor(out=wkv4[:, :, 2], in0=frv[:, :, 0], in1=fr1v[:, :, 1], op=Alu.mult)
                nc.vector.tensor_tensor(out=wkv4[:, :, 3], in0=frv[:, :, 0], in1=frv[:, :, 1], op=Alu.mult)
                ib = spool.tile([128, 2], f32, tag="ib")
                nc.vector.scalar_tensor_tensor(out=ib[:, :], in0=p0v[:, :, 0], scalar=64.0,
                                               in1=p0v[:, :, 1], op0=Alu.mult, op1=Alu.add)
                idxkv = spool.tile([128, 8], f32, tag="idxkv")
                for c in range(2):
                    nc.vector.tensor_tensor(
                        out=idxkv[:, c * 4 : (c + 1) * 4],
                        in0=ib[:, c : c + 1].to_broadcast([128, 4]),
                        in1=kvoff4, op=Alu.add,
                    )
                idxkv_i = spool.tile([128, 8], i32, tag="idxkvi")
                nc.vector.tensor_copy(out=idxkv_i[:, :], in_=idxkv[:, :])
                G = spool.tile([128, 8, 64], f32, tag="G", bufs=1)
                for j in range(4):
                    nc.gpsimd.indirect_dma_start(
                        out=G[:, j, :], out_offset=None, in_=kvT_ap,
                        in_offset=IndirectOffsetOnAxis(ap=idxkv_i[:, j : j + 1], axis=0),
                    )
                if blk == 0:
                    fc["pend0"] = issue_wpair(idxw_i, 0, 0)
                yield
                for j in range(4, 8):
                    nc.gpsimd.indirect_dma_start(
                        out=G[:, j, :], out_offset=None, in_=kvT_ap,
                        in_offset=IndirectOffsetOnAxis(ap=idxkv_i[:, j : j + 1], axis=0),
                    )
                # diag weight matrices: d0 <- fx, d1 <- 1-fx (per chunk)
                diags = []
                for c in range(2):
                    d0 = spool.tile([128, 128], bf16, tag=f"d0_{c}")
                    d1 = spool.tile([128, 128], bf16, tag=f"d1_{c}")
                    nc.vector.tensor_scalar(out=d0[:, :], in0=eye,
                                            scalar1=fr[:, c * 2 + 1 : c * 2 + 2],
                                            scalar2=None, op0=Alu.mult)
                    nc.vector.tensor_scalar(out=d1[:, :], in0=eye,
                                            scalar1=fr1[:, c * 2 + 1 : c * 2 + 2],
                                            scalar2=None, op0=Alu.mult)
                    diags.append((d0, d1))
                fc["diags"] = diags
                yield

                # ---- gathered kv -> xs -> k (per chunk; chunk 0 unblocks QK) ----
                xs_b = spool.tile([65, 256], bf16, tag="xsb")
                nc.vector.memset(xs_b[64:65, :], 1.0)
                k_b = spool.tile([128, 256], bf16, tag="kb")
                vT1 = spool.tile([128, 256], bf16, tag="vT1")
                for c in range(2):
                    xsT = spool.tile([128, 64], f32, tag="xsT")
                    nc.vector.tensor_scalar(
                        out=xsT[:, :], in0=G[:, c * 4 + 0, :],
                        scalar1=wkv[:, c * 4 : c * 4 + 1], scalar2=None, op0=Alu.mult,
                    )
                    for t in range(1, 4):
                        nc.vector.scalar_tensor_tensor(
                            out=xsT[:, :], in0=G[:, c * 4 + t, :],
                            scalar=wkv[:, c * 4 + t : c * 4 + t + 1], in1=xsT[:, :],
                            op0=Alu.mult, op1=Alu.add,
                        )
                    xsp = mps.tile([64, 128], f32, tag="m")
                    nc.tensor.transpose(out=xsp[:, :], in_=xsT[:, :], identity=eye)
                    nc.scalar.activation(out=xs_b[0:64, c * 128 : (c + 1) * 128],
                                         in_=xsp[:, :], func=Act.Copy)
                    kp = mps.tile([128, 128], f32, tag="m")
                    nc.tensor.matmul(out=kp[:, :],
                                     lhsT=wpb[0:65, blk * 192 : blk * 192 + 128],
                                     rhs=xs_b[:, c * 128 : (c + 1) * 128],
                                     start=True, stop=True)
                    nc.scalar.activation(out=k_b[:, c * 128 : (c + 1) * 128],
                                         in_=kp[:, :], func=Act.Copy)
                    if c == 0:
                        fc["k_b"] = k_b
                        yield
                yield

                # ---- v projection ----
                nc.vector.tensor_copy(out=vT1[:, :], in_=vtm[:, :])
                for c in range(2):
                    vp = mps.tile([128, 64], f32, tag="m")
                    nc.tensor.matmul(
                        out=vp[:, :], lhsT=xs_b[:, c * 128 : (c + 1) * 128],
                        rhs=wpb[0:65, blk * 192 + 128 : blk * 192 + 192],
                        start=True, stop=True,
                    )
                    vv = vT1[:, c * 128 : (c + 1) * 128].rearrange("p (h q) -> p h q", q=32)
                    nc.scalar.activation(
                        out=vv[:, :, 0:16],
                        in_=vp[:, :].rearrange("p (h q) -> p h q", q=16),
                        func=Act.Copy,
                    )
                fc["vT1"] = vT1
                yield

            # ======================= attention =======================
            def attn(blk, fc, R, feeder, out_d=None,
                     pend_in=None, next_fc=None):
                po_wT_sp = cpb[:, 128 + blk * 64 : 128 + (blk + 1) * 64]
                b4 = cpb[:, 0:128]
                po_b_hi = cp[0:64, 524 + blk : 525 + blk]
                q_b = fc["q_b"]
                k_b = fc["k_b"]
                vT1 = fc["vT1"]
                idxw_i = fc["idxw_i"]
                fr1 = fc["fr1"]
                diags = fc["diags"]

                steps = [(h, c) for h in range(4) for c in range(2)]

                def issue_gather(idxsrc, i):
                    h, c = steps[i]
                    return issue_wpair(idxsrc, h, c)

                # ---- sw-pipelined tail, interleaved into the last head ----
                sbps = [None] * 8

                def tail_a(j):
                    sbp = tlps.tile([128, 512], f32, tag="tl")
                    nc.tensor.matmul(out=sbp[:, :], lhsT=b4,
                                     rhs=avs[:, j * 512 : (j + 1) * 512],
                                     start=True, stop=True)
                    sbps[j] = sbp

                def tail_b(j):
                    rcp = spool.tile([128, 512], f32, tag="rcp")
                    act_raw(rcp[:, :], sbps[j][:, :], Act.Reciprocal)
                    on = spool.tile([128, 512], bf16, tag="on", bufs=1)
                    nc.vector.tensor_tensor(out=on[:, :],
                                            in0=avs[:, j * 512 : (j + 1) * 512],
                                            in1=rcp[:, :], op=Alu.mult)
                    op = tlps.tile([128, 512], f32, tag="tl")
                    nc.tensor.matmul(out=op[0:64, :], lhsT=po_wT_sp, rhs=on[:, :],
                                     start=True, stop=True)
                    nc.vector.scalar_tensor_tensor(
                        out=R[0:64, j * 512 : (j + 1) * 512], in0=op[0:64, :],
                        scalar=po_b_hi, in1=R[0:64, j * 512 : (j + 1) * 512],
                        op0=Alu.add, op1=Alu.add,
                    )
                    if out_d is not None:
                        nc.sync.dma_start(
                            out=out_d[64:128, j * 512 : (j + 1) * 512],
                            in_=R[0:64, j * 512 : (j + 1) * 512],
                        )

                avs = apool.tile([128, HWS], bf16, tag="avs")
                pend = dict(pend_in) if pend_in else {}
                if 0 not in pend and "pend0" in fc:
                    pend[0] = fc.pop("pend0")
                for i0 in range(2):
                    if i0 not in pend:
                        pend[i0] = issue_gather(idxw_i, i0)
                pend_next = {}

                def compute_y(j):
                    cj = steps[j][1]
                    Wt, Dw = pend.pop(j)
                    Y = wpool.tile([128, 4160], bf16, tag="Y")
                    nc.vector.tensor_scalar(out=Y[:, :], in0=Dw[:, :],
                                            scalar1=fr1[:, cj * 2 : cj * 2 + 1],
                                            scalar2=None, op0=Alu.mult)
                    nc.vector.tensor_tensor(out=Y[:, :], in0=Y[:, :], in1=Wt[:, :],
                                            op=Alu.add)
                    return Y

                ys = {0: compute_y(0)}
                P = None
                for i, (h, c) in enumerate(steps):
                    if c == 0:
                        P = ppool.tile([128, 2, HWS], bf16, tag="P")
                    Y = ys.pop(i)
                    if i + 2 < 8:
                        pend[i + 2] = issue_gather(idxw_i, i + 2)
                    elif next_fc is not None:
                        # prefetch next block's first gather pairs
                        while "idxw_i" not in next_fc:
                            if not feeder():
                                break
                        if "idxw_i" in next_fc:
                            pend_next[i - 6] = issue_gather(next_fc["idxw_i"], i - 6)
                    if i + 1 < 8:
                        ys[i + 1] = compute_y(i + 1)
                    Y3 = Y[:, :].rearrange("p (r q) -> p r q", q=65)
                    kh = k_b[h * 32 : h * 32 + 17, c * 128 : (c + 1) * 128]
                    d0, d1 = diags[c]
                    for k in range(4):
                        qk = qkps.tile([128, 1024], f32, tag="qk")
                        for hf in range(2):
                            mc = k * 2 + hf
                            nc.tensor.matmul(
                                out=qk[:, hf * 512 : (hf + 1) * 512], lhsT=kh,
                                rhs=q_b[h * 32 : h * 32 + 17, mc * 512 : (mc + 1) * 512],
                                start=True, stop=False, tile_position=(h * 32, 0),
                            )
                        for hf in range(2):
                            mc = k * 2 + hf
                            nc.tensor.matmul(
                                out=qk[:, hf * 512 : (hf + 1) * 512], lhsT=d0,
                                rhs=Y3[:, mc * 8 : (mc + 1) * 8, 0:64],
                                start=False, stop=False,
                            )
                        for hf in range(2):
                            mc = k * 2 + hf
                            nc.tensor.matmul(
                                out=qk[:, hf * 512 : (hf + 1) * 512], lhsT=d1,
                                rhs=Y3[:, mc * 8 : (mc + 1) * 8, 1:65],
                                start=False, stop=True,
                            )
                        nc.scalar.activation(
                            out=P[:, c, k * 1024 : (k + 1) * 1024], in_=qk[:, :],
                            func=Act.Exp, bias=zb[:, :],
                        )
                    feeder()
                    if c == 1:
                        # AV for this head (+ tail interleaved into head 3)
                        for pr in range(4):
                            a0 = tlps.tile([128, 512], f32, tag="tl")
                            a1 = tlps.tile([128, 512], f32, tag="tl")
                            mca, mcb = pr * 2, pr * 2 + 1
                            for cc in range(2):
                                lw = vT1[:, cc * 128 + h * 32 : cc * 128 + (h + 1) * 32]
                                nc.tensor.matmul(
                                    out=a0[0:32, :], lhsT=lw,
                                    rhs=P[:, cc, mca * 512 : (mca + 1) * 512],
                                    start=(cc == 0), stop=(cc == 1),
                                )
                                nc.tensor.matmul(
                                    out=a1[0:32, :], lhsT=lw,
                                    rhs=P[:, cc, mcb * 512 : (mcb + 1) * 512],
                                    start=(cc == 0), stop=(cc == 1),
                                )
                            nc.vector.tensor_copy(
                                out=avs[h * 32 : (h + 1) * 32, mca * 512 : (mca + 1) * 512],
                                in_=a0[0:32, :])
                            nc.vector.tensor_copy(
                                out=avs[h * 32 : (h + 1) * 32, mcb * 512 : (mcb + 1) * 512],
                                in_=a1[0:32, :])
                            if h == 3:
                                tail_a(pr * 2)
                                tail_a(pr * 2 + 1)
                                if pr >= 1:
                                    tail_b(pr * 2 - 2)
                                    tail_b(pr * 2 - 1)
                        feeder()
                tail_b(6)
                tail_b(7)
                return pend_next

            def make_feeder(gen):
                def feeder():
                    if gen is None:
                        return False
                    try:
                        next(gen)
                        return True
                    except StopIteration:
                        return False
                return feeder

            def drain(gen):
                for _ in gen:
                    pass

            # ======================= schedule =======================
            fc0 = {}
            g0 = front(0, None, kvT0_d, fc0, stream_d=xq1_d)
            next(g0)
            load_bulk()
            drain(g0)
            nc.sync.dma_start(out=o1_d[0:64, :], in_=xi1_d[0:64, :])
            nc.sync.dma_start(out=o2_d[0:64, :], in_=xi2_d[0:64, :])
            fc1 = {}
            g1 = front(1, xq2[0:64, :], kvT0_d, fc1)
            pend1 = attn(0, fc0, xr1, make_feeder(g1), out_d=o1_d,
                         next_fc=fc1)
            drain(g1)
            fc2 = {}
            g2 = front(2, xq2[0:64, :], kvT1_d, fc2)
            pend2 = attn(1, fc1, xr2, make_feeder(g2),
                         pend_in=pend1 or None, next_fc=fc2)
            drain(g2)
            attn(2, fc2, xr2, make_feeder(None), out_d=o2_d,
                 pend_in=pend2 or None)

    nc.compile()
    return nc


def _host_prep(inputs):
    """Build per-core in_maps. inputs: dict of full numpy arrays."""
    import ml_dtypes

    x0, x1, x2 = inputs["x0"], inputs["x1"], inputs["x2"]

    def spread_cols(m):
        # m: [64(in), 64(out)] -> [64(in), 128] with out col h*16+j at h*32+j
        out = np.zeros((m.shape[0], 128), m.dtype)
        for h in range(4):
            out[:, h * 32 : h * 32 + 16] = m[:, h * 16 : (h + 1) * 16]
        return out

    def spread_rows(v):
        # v: [64, k] -> [128, k] with row h*16+j at h*32+j
        out = np.zeros((128,) + v.shape[1:], v.dtype)
        for h in range(4):
            out[h * 32 : h * 32 + 16] = v[h * 16 : (h + 1) * 16]
        return out

    # weight pack bf16: [64, 3*128]  (spread pq_wT)
    wpf = np.zeros((64, 3 * 128), np.float32)
    for b in range(3):
        wpf[:, b * 128 : (b + 1) * 128] = spread_cols(inputs["pq_w"][b].T)
    wpb = np.zeros((65, 3 * 192), ml_dtypes.bfloat16)
    for b in range(3):
        o = b * 192
        pk = np.zeros((65, 128), np.float32)
        pk[0:64] = spread_cols(inputs["pk_w"][b].T * 0.25)
        for h in range(4):
            pk[64, h * 32 : h * 32 + 16] = inputs["pk_b"][b][h * 16 : (h + 1) * 16] * 0.25
        for h in range(4):
            pq_bh = inputs["pq_b"][b][h * 16 : (h + 1) * 16]
            pk[:, h * 32 + 16] = pk[:, h * 32 : h * 32 + 16] @ pq_bh
        wpb[:, o : o + 128] = pk.astype(ml_dtypes.bfloat16)
        wpb[:64, o + 128 : o + 192] = inputs["pv_w"][b].T.astype(ml_dtypes.bfloat16)
        wpb[64, o + 128 : o + 192] = inputs["pv_b"][b].astype(ml_dtypes.bfloat16)
    # const pack [128, 598]
    cp = np.zeros((128, 599), np.float32)
    for h in range(4):
        cp[h * 32 + 16, 598] = 1.0
    cp[:, 0:128] = np.eye(128, dtype=np.float32)
    ys = (np.linspace(0.5, HK - 0.5, HK) / (HK - 1.0)) * 2.0 - 1.0
    cp[0, 128:384] = np.repeat(ys, WK)         # y per n (i-major)
    cp[1, 128:384] = np.tile(ys, HK)           # x per n
    cp[0, 384:512] = 1.0                       # ones1_128
    for h in range(4):
        cp[h * 32 : h * 32 + 16, 520] = 1.0 / 64.0
    for b in range(3):
        cp[:, 521 + b] = spread_rows(inputs["pq_b"][b][:, None])[:, 0]
        cp[0:64, 524 + b] = inputs["po_b"][b]
        bc0 = 527 + b * 21
        cp[:, bc0 : bc0 + 16] = spread_rows(inputs["dw_w"][b].reshape(64, 16))
        dwb_eff = (inputs["dw_b"][b]
                   + inputs["dw_w"][b].reshape(64, 16).sum(1) * inputs["pq_b"][b])
        cp[:, bc0 + 16] = spread_rows(dwb_eff[:, None])[:, 0]
        cp[:, bc0 + 17] = spread_rows(inputs["ln_g"][b][:, None])[:, 0]
        cp[:, bc0 + 18] = spread_rows(inputs["ln_b"][b][:, None])[:, 0]
        cp[:, bc0 + 19 : bc0 + 21] = spread_rows(inputs["pw_w"][b].T)
    cp[:, 590] = 0.0
    cp[:, 591] = 1.0
    cp[:, 592] = 64.0
    cp[:, 593] = 65.0
    for h in range(4):
        cp[:, 594 + h] = float(h * THEAD)
    cpb = np.zeros((128, 320), ml_dtypes.bfloat16)
    b4 = np.zeros((128, 128), np.float32)
    for h in range(4):
        b4[h * 32 + 16, h * 32 : (h + 1) * 32] = 1.0
    cpb[:, 0:128] = b4.astype(ml_dtypes.bfloat16)
    for b in range(3):
        poT = inputs["po_w"][b].T  # [c, o]
        for h in range(4):
            cpb[h * 32 : h * 32 + 16, 128 + b * 64 : 128 + (b + 1) * 64] = poT[
                h * 16 : (h + 1) * 16
            ].astype(ml_dtypes.bfloat16)
    # rpe slice tables bf16: T windows then D (row-diff) windows
    tab = np.zeros((2, NBLK, NH, 64, TROW, TCOL), ml_dtypes.bfloat16)
    rpe = inputs["rpe"]
    for b in range(3):
        for h in range(4):
            pad = np.zeros((129, 128), np.float32)
            pad[0:127, 0:127] = rpe[b, h]
            dif = pad[1:129] - pad[0:128]
            for x0s in range(64):
                tab[0, b, h, x0s] = pad[0:128, x0s : x0s + 65].astype(ml_dtypes.bfloat16)
                tab[1, b, h, x0s] = dif[:, x0s : x0s + 65].astype(ml_dtypes.bfloat16)
    tab = tab.reshape(-1, 1)

    in_maps = []
    for bb in range(B):
        m = {
            "xi1": np.ascontiguousarray(x1[bb].reshape(C, HWS)),
            "xi2": np.ascontiguousarray(x2[bb].reshape(C, HWS)),
            "kvT0": np.ascontiguousarray(x0[bb, :64].reshape(64, HWS).T),
            "xq1": np.ascontiguousarray(x1[bb, :64].reshape(64, HWS)),
            "xq2": np.ascontiguousarray(x2[bb, :64].reshape(64, HWS)),
            "kvT1": np.ascontiguousarray(x1[bb, :64].reshape(64, HWS).T),
            "wpf": wpf,
            "wpb": wpb,
            "cp": cp,
            "cpb": cpb,
            "rpetab": tab,
        }
        in_maps.append(m)
    return in_maps


def kernel(**inputs):
    from concourse.bass_utils import run_bass_kernel_spmd

    if "nc" not in _CACHE:
        _CACHE["nc"] = _build_graph()
    nc = _CACHE["nc"]
    in_maps = _host_prep(inputs)
    res = run_bass_kernel_spmd(nc, in_maps, core_ids=list(range(8)))
    out = np.zeros((NBLK, B, C, H, W), np.float32)
    out[0] = inputs["x0"]
    for bb in range(B):
        out[1, bb] = res.results[bb]["o1"].reshape(C, H, W)
        out[2, bb] = res.results[bb]["o2"].reshape(C, H, W)
    return out


# revision 48
# speedup vs baseline: 1.1084x; 1.0100x over previous
"""Trainium2 Bass kernel for nn_AttentionTD (3-block deformable attention TD).

Self-contained: hardcodes all shapes. Data-parallel over batch B=8 across the
8 NeuronCores; each core runs the full 3-block DAT stack for one batch element.

Pipelined emission: block b+1's front-end (q-proj, offset conv, LN, GELU,
index math, kv gather/proj) is interleaved under block b's attention so the
tensor engine never drains between blocks.
"""

import sys

sys.path.insert(0, "/opt/trn_rl_repo")

import numpy as np

# ---------------- problem constants ----------------
B, C, H, W = 8, 128, 64, 64
NCH = 64          # channels per DAT block
NH, HC = 4, 16    # heads, head channels
KS = 4
HWS = H * W       # 4096
HK = WK = 16
NS = HK * WK      # 256 sample points
EPS = 1e-5
NBLK = 3
# rpe slice table geometry: [blk][h][x0 (64)][row (128)][col (65)]
TROW, TCOL = 128, 65
TSLICE = TROW * TCOL          # 8320
THEAD = 64 * TSLICE           # per (blk,h)
TBLK = NH * THEAD
NTAB = NBLK * TBLK

_CACHE = {}


def _build_graph():
    from concourse import bacc, mybir, tile
    import concourse.bass as bass
    from concourse.bass import IndirectOffsetOnAxis

    f32 = mybir.dt.float32
    bf16 = mybir.dt.bfloat16
    i32 = mybir.dt.int32
    Alu = mybir.AluOpType
    Act = mybir.ActivationFunctionType

    nc = bacc.Bacc("TRN2", target_bir_lowering=False, debug=False, num_devices=8)

    # ---- dram io ----
    xi1_d = nc.dram_tensor("xi1", [C, HWS], f32, kind="ExternalInput").ap()
    xi2_d = nc.dram_tensor("xi2", [C, HWS], f32, kind="ExternalInput").ap()
    kvT0_d = nc.dram_tensor("kvT0", [HWS, NCH], f32, kind="ExternalInput").ap()
    kvT1_d = nc.dram_tensor("kvT1", [HWS, NCH], f32, kind="ExternalInput").ap()
    xq1_d = nc.dram_tensor("xq1", [64, HWS], mybir.dt.float32r, kind="ExternalInput").ap()
    xq2_d = nc.dram_tensor("xq2", [64, HWS], mybir.dt.float32r, kind="ExternalInput").ap()
    wpf_d = nc.dram_tensor("wpf", [64, 3 * 128], mybir.dt.float32r, kind="ExternalInput").ap()
    wpb_d = nc.dram_tensor("wpb", [65, 3 * 192], bf16, kind="ExternalInput").ap()
    cp_d = nc.dram_tensor("cp", [128, 599], f32, kind="ExternalInput").ap()
    cpb_d = nc.dram_tensor("cpb", [128, 320], bf16, kind="ExternalInput").ap()
    tab_d = nc.dram_tensor("rpetab", [2 * NTAB, 1], bf16, kind="ExternalInput").ap()
    o1_d = nc.dram_tensor("o1", [C, HWS], f32, kind="ExternalOutput").ap()
    o2_d = nc.dram_tensor("o2", [C, HWS], f32, kind="ExternalOutput").ap()

    with tile.TileContext(nc) as tc:
        import contextlib

        ctx = contextlib.ExitStack()
        with ctx:
            cpool = ctx.enter_context(tc.tile_pool(name="const", bufs=1))
            xpool = ctx.enter_context(tc.tile_pool(name="xdata", bufs=1))
            qpool = ctx.enter_context(tc.tile_pool(name="qtiles", bufs=2))
            wpool = ctx.enter_context(tc.tile_pool(name="wins", bufs=2))
            ppool = ctx.enter_context(tc.tile_pool(name="probs", bufs=1))
            apool = ctx.enter_context(tc.tile_pool(name="avsp", bufs=1))
            spool = ctx.enter_context(tc.tile_pool(name="small", bufs=2))
            qkps = ctx.enter_context(tc.tile_pool(name="qk", bufs=2, space="PSUM"))
            tlps = ctx.enter_context(tc.tile_pool(name="tl", bufs=3, space="PSUM"))
            mps = ctx.enter_context(tc.tile_pool(name="misc", bufs=1, space="PSUM"))

            # ---- persistent loads ----
            cp = cpool.tile([128, 599], f32, tag="cp")
            nc.sync.dma_start(out=cp[:, :], in_=cp_d)
            wpf = cpool.tile([64, 3 * 128], mybir.dt.float32r, tag="wpf")
            nc.sync.dma_start(out=wpf[:, :], in_=wpf_d)
            wpb = cpool.tile([65, 3 * 192], bf16, tag="wpb")
            nc.sync.dma_start(out=wpb[:, :], in_=wpb_d)
            cpb = cpool.tile([128, 320], bf16, tag="cpb")
            xq2 = xpool.tile([64, HWS], mybir.dt.float32r, tag="xq2")
            xr1 = xpool.tile([64, HWS], f32, tag="xr1")
            xr2 = xpool.tile([64, HWS], f32, tag="xr2")

            def load_bulk():
                nc.sync.dma_start(out=cpb[:, :], in_=cpb_d)
                nc.sync.dma_start(out=xq2[:, :], in_=xq2_d)
                nc.sync.dma_start(out=xr1[:, :], in_=xi1_d[64:128, :])
                nc.sync.dma_start(out=xr2[:, :], in_=xi2_d[64:128, :])


            def act_raw(out, in_, func):
                eng = nc.scalar
                ins = [eng.lower_ap(in_)]
                for v in (0.0, 1.0, 0.0):
                    ins.append(mybir.ImmediateValue(dtype=mybir.dt.float32, value=v))
                return eng.add_instruction(
                    mybir.InstActivation(
                        name=nc.get_next_instruction_name(), func=func,
                        ins=ins, outs=[eng.lower_ap(out)],
                    )
                )

            zb = cpool.tile([128, 1], f32, tag="zb")
            nc.vector.memset(zb[:, :], 0.0)
            epst = cpool.tile([1, 1], f32, tag="epst")
            nc.vector.memset(epst[:, :], EPS)
            # vT1 template: zeros with 1.0 at (c*128 + h*32 + 16)
            vtm = cpool.tile([128, 256], bf16, tag="vtm")
            nc.vector.memset(vtm[:, :], 0.0)
            nc.vector.memset(
                vtm[:, :].rearrange("p (c h q) -> p c h q", c=2, q=32)[:, :, :, 16:17],
                1.0,
            )

            eye = cp[:, 0:128]
            ref_yx = cp[0:2, 128:384]          # row0 = y, row1 = x
            ones1_128 = cp[0:1, 384:512]       # [1,128] ones (bcast lhsT)
            ones128_div = cp[0:128, 520:521]   # 1/64 on data rows, 0 on gaps
            kvoff4 = cp[:, 590:594]            # (0,1,64,65) rows
            headoff4 = cp[:, 594:598]          # (0,T,2T,3T) rows
            ones16 = cp[:, 598:599]            # 1.0 at rows h*32+16


            def issue_wpair(idxsrc, h, c):
                Wt = wpool.tile([128, 4160], bf16, tag="Wt", bufs=3, name="Wt")
                nc.gpsimd.indirect_dma_start(
                    out=Wt[:, :], out_offset=None, in_=tab_d,
                    in_offset=IndirectOffsetOnAxis(
                        ap=idxsrc[:, c * 4 + h : c * 4 + h + 1], axis=0),
                )
                Dw = wpool.tile([128, 4160], bf16, tag="Dw", bufs=3, name="Dw")
                nc.gpsimd.indirect_dma_start(
                    out=Dw[:, :], out_offset=None, in_=tab_d,
                    in_offset=IndirectOffsetOnAxis(
                        ap=idxsrc[:, c * 4 + h : c * 4 + h + 1], axis=0),
                    element_offset=NTAB,
                )
                return Wt, Dw

            # ======================= front-end =======================
            def front(blk, XQb, kvT_ap, fc, stream_d=None):
                bc0 = 527 + blk * 21
                dw_w = cp[:, bc0 : bc0 + 16]
                dw_b = cp[:, bc0 + 16 : bc0 + 17]
                ln_g = cp[:, bc0 + 17 : bc0 + 18]
                ln_b = cp[:, bc0 + 18 : bc0 + 19]
                pw_wT = cp[:, bc0 + 19 : bc0 + 21]
                pq_b_sp = cp[:, 521 + blk : 522 + blk]

                # ---- q projection (f32 copy for conv path + bf16 for QK) ----
                q_f = qpool.tile([128, HWS], mybir.dt.float32r, tag="qf", bufs=1)
                q_b = qpool.tile([128, HWS], bf16, tag="qb")
                f32r = mybir.dt.float32r
                for kq in range(4):
                    if stream_d is not None:
                        xqc = spool.tile([64, 1024], mybir.dt.float32r, tag="xqc", bufs=1)
                        nc.sync.dma_start(out=xqc[:, :],
                                          in_=stream_d[:, kq * 1024 : (kq + 1) * 1024])
                        rhs_kq = xqc[:, :]
                    else:
                        rhs_kq = XQb[:, kq * 1024 : (kq + 1) * 1024]
                    qp = qkps.tile([128, 1024], f32, tag="qk")
                    for hf in range(2):
                        nc.tensor.matmul(
                            out=qp[:, hf * 512 : (hf + 1) * 512],
                            lhsT=wpf[:, blk * 128 : (blk + 1) * 128],
                            rhs=rhs_kq[:, hf * 512 : (hf + 1) * 512],
                            start=True, stop=True,
                        )
                    for hf in range(2):
                        nc.scalar.activation(
                            out=q_f[:, kq * 1024 + hf * 512 : kq * 1024 + (hf + 1) * 512],
                            in_=qp[:, hf * 512 : (hf + 1) * 512], func=Act.Copy)
                    nc.vector.tensor_scalar(
                        out=q_b[:, kq * 1024 : (kq + 1) * 1024], in0=qp[:, :],
                        scalar1=ones16, scalar2=None, op0=Alu.add,
                    )
                    if kq == 1:
                        yield
                fc["q_b"] = q_b
                yield
                yield

                # ---- depthwise 4x4 stride-4 conv (f32r diag matmuls) ----
                q5 = q_f[:, :].rearrange("p (hh a ww b) -> p hh a ww b", hh=16, a=4, ww=16, b=4)
                dwp = mps.tile([128, 256], f32, tag="m")
                for t in range(16):
                    dy, dx = t // 4, t % 4
                    dg = spool.tile([128, 128], mybir.dt.float32r, tag="dg", bufs=2)
                    nc.vector.tensor_scalar(out=dg[:, :], in0=eye,
                                            scalar1=dw_w[:, t : t + 1],
                                            scalar2=None, op0=Alu.mult)
                    nc.tensor.matmul(
                        out=dwp[:, :], lhsT=dg[:, :],
                        rhs=q5[:, :, dy, :, dx],
                        start=(t == 0), stop=(t == 15),
                    )
                    if t == 8:
                        yield
                accp = spool.tile([128, 512], f32, tag="accp", bufs=1)
                acc = accp[:, 0:256]
                nc.vector.tensor_scalar(out=acc, in0=dwp[:, :], scalar1=dw_b,
                                        scalar2=None, op0=Alu.add)
                nc.vector.tensor_tensor(out=accp[:, 256:512], in0=acc,
                                        in1=acc, op=Alu.mult)
                yield

                # ---- layernorm stats ----
                stp = mps.tile([1, 512], f32, tag="m")
                nc.tensor.matmul(out=stp[:, :], lhsT=ones128_div, rhs=accp[:, :],
                                 start=True, stop=True)
                stats = spool.tile([1, 512], f32, tag="stats")
                nc.vector.tensor_copy(out=stats[:, 0:256], in_=stp[:, 0:256])
                var = spool.tile([1, 256], f32, tag="var")
                nc.vector.tensor_tensor(out=var[:, :], in0=stats[:, 0:256],
                                        in1=stats[:, 0:256], op=Alu.mult)
                nc.vector.tensor_tensor(out=var[:, :], in0=stp[:, 256:512],
                                        in1=var[:, :], op=Alu.subtract)
                nc.scalar.activation(out=stats[:, 256:512], in_=var[:, :],
                                     func=Act.Abs_reciprocal_sqrt, bias=epst[:, :])
                yield

                # ---- normalize + gelu ----
                bcp = mps.tile([128, 512], f32, tag="m")
                nc.tensor.matmul(out=bcp[:, :], lhsT=ones1_128, rhs=stats[:, :],
                                 start=True, stop=True)
                t1 = spool.tile([128, 256], f32, tag="t1")
                nc.vector.tensor_tensor(out=t1[:, :], in0=accp[:, 0:256],
                                        in1=bcp[:, 0:256], op=Alu.subtract)
                nc.vector.tensor_tensor(out=t1[:, :], in0=t1[:, :],
                                        in1=bcp[:, 256:512], op=Alu.mult)
                nc.vector.tensor_scalar(out=t1[:, :], in0=t1[:, :], scalar1=ln_g,
                                        scalar2=ln_b, op0=Alu.mult, op1=Alu.add)
                gl = spool.tile([128, 256], f32, tag="gl")
                nc.scalar.activation(out=gl[:, :], in_=t1[:, :], func=Act.Gelu)
                yield

                # ---- offsets -> positions -> posT ----
                offp = mps.tile([2, 256], f32, tag="m")
                nc.tensor.matmul(out=offp[:, :], lhsT=pw_wT, rhs=gl[:, :],
                                 start=True, stop=True)
                pos = spool.tile([2, 256], f32, tag="pos")
                nc.vector.tensor_tensor(out=pos[:, :], in0=offp[:, :], in1=ref_yx, op=Alu.add)
                nc.vector.tensor_scalar(out=pos[:, :], in0=pos[:, :], scalar1=1.0,
                                        scalar2=-1.0, op0=Alu.min, op1=Alu.max)
                posT = spool.tile([128, 4], f32, tag="posT")  # (c0y c0x c1y c1x)
                for c in range(2):
                    tp = mps.tile([128, 2], f32, tag="m")
                    nc.tensor.transpose(out=tp[:, :], in_=pos[:, c * 128 : (c + 1) * 128],
                                        identity=eye[0:2, 0:2])
                    nc.vector.tensor_copy(out=posT[:, c * 2 : c * 2 + 2], in_=tp[:, :])
                yield

                # ---- batched index math ----
                pix = spool.tile([128, 4], f32, tag="pix")
                nc.vector.tensor_scalar(out=pix[:, :], in0=posT[:, :], scalar1=1.0,
                                        scalar2=31.5, op0=Alu.add, op1=Alu.mult)
                rnd = spool.tile([128, 4], f32, tag="rnd")
                nc.vector.tensor_scalar(out=rnd[:, :], in0=pix[:, :], scalar1=8388608.0,
                                        scalar2=-8388608.0, op0=Alu.add, op1=Alu.add)
                gt = spool.tile([128, 4], f32, tag="gt")
                nc.vector.tensor_tensor(out=gt[:, :], in0=rnd[:, :], in1=pix[:, :], op=Alu.is_gt)
                p0 = spool.tile([128, 4], f32, tag="p0")
                nc.vector.tensor_tensor(out=p0[:, :], in0=rnd[:, :], in1=gt[:, :], op=Alu.subtract)
                nc.vector.tensor_scalar(out=p0[:, :], in0=p0[:, :], scalar1=62.0,
                                        scalar2=None, op0=Alu.min)
                fr = spool.tile([128, 4], f32, tag="fr")
                nc.vector.tensor_tensor(out=fr[:, :], in0=pix[:, :], in1=p0[:, :], op=Alu.subtract)
                fr1 = spool.tile([128, 4], f32, tag="fr1")
                nc.vector.tensor_scalar(out=fr1[:, :], in0=fr[:, :], scalar1=-1.0,
                                        scalar2=1.0, op0=Alu.mult, op1=Alu.add)
                fc["fr"] = fr
                fc["fr1"] = fr1
                yield

                # chunk views: v=0 -> y, v=1 -> x
                p0v = p0[:, :].rearrange("p (c v) -> p c v", v=2)
                frv = fr[:, :].rearrange("p (c v) -> p c v", v=2)
                fr1v = fr1[:, :].rearrange("p (c v) -> p c v", v=2)

                # ---- bias-window indices (early, enables W prefetch) ----
                q0b = spool.tile([128, 4], f32, tag="q0b")
                nc.vector.tensor_scalar(out=q0b[:, :], in0=p0[:, :], scalar1=-1.0,
                                        scalar2=62.0, op0=Alu.mult, op1=Alu.add)
                q0bv = q0b[:, :].rearrange("p (c v) -> p c v", v=2)
                iw = spool.tile([128, 2], f32, tag="iw")
                nc.vector.scalar_tensor_tensor(out=iw[:, :], in0=q0bv[:, :, 1], scalar=128.0,
                                               in1=q0bv[:, :, 0], op0=Alu.mult, op1=Alu.add)
                nc.vector.tensor_scalar(out=iw[:, :], in0=iw[:, :], scalar1=65.0,
                                        scalar2=float(blk * TBLK), op0=Alu.mult, op1=Alu.add)
                idxw = spool.tile([128, 8], f32, tag="idxw")
                for c in range(2):
                    nc.vector.tensor_tensor(
                        out=idxw[:, c * 4 : (c + 1) * 4],
                        in0=iw[:, c : c + 1].to_broadcast([128, 4]),
                        in1=headoff4, op=Alu.add,
                    )
                idxw_i = spool.tile([128, 8], i32, tag="idxwi")
                nc.vector.tensor_copy(out=idxw_i[:, :], in_=idxw[:, :])
                fc["idxw_i"] = idxw_i
                yield

                # ---- kv bilinear weights + gather indices ----
                wkv = spool.tile([128, 8], f32, tag="wkv")
                wkv4 = wkv[:, :].rearrange("p (c t) -> p c t", t=4)
                nc.vector.tensor_tensor(out=wkv4[:, :, 0], in0=fr1v[:, :, 0], in1=fr1v[:, :, 1], op=Alu.mult)
                nc.vector.tensor_tensor(out=wkv4[:, :, 1], in0=fr1v[:, :, 0], in1=frv[:, :, 1], op=Alu.mult)
                nc.vector.tensor_tensor(out=wkv4[:, :, 2], in0=frv[:, :, 0], in1=fr1v[:, :, 1], op=Alu.mult)
                nc.vector.tensor_tensor(out=wkv4[:, :, 3], in0=frv[:, :, 0], in1=frv[:, :, 1], op=Alu.mult)
                ib = spool.tile([128, 2], f32, tag="ib")
                nc.vector.scalar_tensor_tensor(out=ib[:, :], in0=p0v[:, :, 0], scalar=64.0,
                                               in1=p0v[:, :, 1], op0=Alu.mult, op1=Alu.add)
                idxkv = spool.tile([128, 8], f32, tag="idxkv")
                for c in range(2):
                    nc.vector.tensor_tensor(
                        out=idxkv[:, c * 4 : (c + 1) * 4],
                        in0=ib[:, c : c + 1].to_broadcast([128, 4]),
                        in1=kvoff4, op=Alu.add,
                    )
                idxkv_i = spool.tile([128, 8], i32, tag="idxkvi")
                nc.vector.tensor_copy(out=idxkv_i[:, :], in_=idxkv[:, :])
                G = spool.tile([128, 8, 64], f32, tag="G", bufs=1)
                for j in range(4):
                    nc.gpsimd.indirect_dma_start(
                        out=G[:, j, :], out_offset=None, in_=kvT_ap,
                        in_offset=IndirectOffsetOnAxis(ap=idxkv_i[:, j : j + 1], axis=0),
                    )
                if blk == 0:
                    fc["pend0"] = issue_wpair(idxw_i, 0, 0)
                yield
                for j in range(4, 8):
                    nc.gpsimd.indirect_dma_start(
                        out=G[:, j, :], out_offset=None, in_=kvT_ap,
                        in_offset=IndirectOffsetOnAxis(ap=idxkv_i[:, j : j + 1], axis=0),
                    )
                # diag weight matrices: d0 <- fx, d1 <- 1-fx (per chunk)
                diags = []
                for c in range(2):
                    d0 = spool.tile([128, 128], bf16, tag=f"d0_{c}")
                    d1 = spool.tile([128, 128], bf16, tag=f"d1_{c}")
                    nc.vector.tensor_scalar(out=d0[:, :], in0=eye,
                                            scalar1=fr[:, c * 2 + 1 : c * 2 + 2],
                                            scalar2=None, op0=Alu.mult)
                    nc.vector.tensor_scalar(out=d1[:, :], in0=eye,
                                            scalar1=fr1[:, c * 2 + 1 : c * 2 + 2],
                                            scalar2=None, op0=Alu.mult)
                    diags.append((d0, d1))
                fc["diags"] = diags
                yield

                # ---- gathered kv -> xs -> k (per chunk; chunk 0 unblocks QK) ----
                xs_b = spool.tile([65, 256], bf16, tag="xsb")
                nc.vector.memset(xs_b[64:65, :], 1.0)
                k_b = spool.tile([128, 256], bf16, tag="kb")
                vT1 = spool.tile([128, 256], bf16, tag="vT1")
                for c in range(2):
                    xsT = spool.tile([128, 64], f32, tag="xsT")
                    nc.vector.tensor_scalar(
                        out=xsT[:, :], in0=G[:, c * 4 + 0, :],
                        scalar1=wkv[:, c * 4 : c * 4 + 1], scalar2=None, op0=Alu.mult,
                    )
                    for t in range(1, 4):
                        nc.vector.scalar_tensor_tensor(
                            out=xsT[:, :], in0=G[:, c * 4 + t, :],
                            scalar=wkv[:, c * 4 + t : c * 4 + t + 1], in1=xsT[:, :],
                            op0=Alu.mult, op1=Alu.add,
                        )
                    xsp = mps.tile([64, 128], f32, tag="m")
                    nc.tensor.transpose(out=xsp[:, :], in_=xsT[:, :], identity=eye)
                    nc.scalar.activation(out=xs_b[0:64, c * 128 : (c + 1) * 128],
                                         in_=xsp[:, :], func=Act.Copy)
                    kp = mps.tile([128, 128], f32, tag="m")
                    nc.tensor.matmul(out=kp[:, :],
                                     lhsT=wpb[0:65, blk * 192 : blk * 192 + 128],
                                     rhs=xs_b[:, c * 128 : (c + 1) * 128],
                                     start=True, stop=True)
                    nc.scalar.activation(out=k_b[:, c * 128 : (c + 1) * 128],
                                         in_=kp[:, :], func=Act.Copy)
                    if c == 0:
                        fc["k_b"] = k_b
                        yield
                yield

                # ---- v projection ----
                nc.vector.tensor_copy(out=vT1[:, :], in_=vtm[:, :])
                for c in range(2):
                    vp = mps.tile([128, 64], f32, tag="m")
                    nc.tensor.matmul(
                        out=vp[:, :], lhsT=xs_b[:, c * 128 : (c + 1) * 128],
                        rhs=wpb[0:65, blk * 192 + 128 : blk * 192 + 192],
                        start=True, stop=True,
                    )
                    vv = vT1[:, c * 128 : (c + 1) * 128].rearrange("p (h q) -> p h q", q=32)
                    nc.scalar.activation(
                        out=vv[:, :, 0:16],
                        in_=vp[:, :].rearrange("p (h q) -> p h q", q=16),
                        func=Act.Copy,
                    )
                fc["vT1"] = vT1
                yield

            # ======================= attention =======================
            def attn(blk, fc, R, feeder, out_d=None,
                     pend_in=None, next_fc=None):
                po_wT_sp = cpb[:, 128 + blk * 64 : 128 + (blk + 1) * 64]
                b4 = cpb[:, 0:128]
                po_b_hi = cp[0:64, 524 + blk : 525 + blk]
                q_b = fc["q_b"]
                k_b = fc["k_b"]
                vT1 = fc["vT1"]
                idxw_i = fc["idxw_i"]
                fr1 = fc["fr1"]
                diags = fc["diags"]

                steps = [(h, c) for h in range(4) for c in range(2)]

                def issue_gather(idxsrc, i):
                    h, c = steps[i]
                    return issue_wpair(idxsrc, h, c)

                # ---- sw-pipelined tail, interleaved into the last head ----
                sbps = [None] * 8

                def tail_a(j):
                    sbp = tlps.tile([128, 512], f32, tag="tl")
                    nc.tensor.matmul(out=sbp[:, :], lhsT=b4,
                                     rhs=avs[:, j * 512 : (j + 1) * 512],
                                     start=True, stop=True)
                    sbps[j] = sbp

                def tail_b(j):
                    rcp = spool.tile([128, 512], f32, tag="rcp")
                    act_raw(rcp[:, :], sbps[j][:, :], Act.Reciprocal)
                    on = spool.tile([128, 512], bf16, tag="on", bufs=1)
                    nc.vector.tensor_tensor(out=on[:, :],
                                            in0=avs[:, j * 512 : (j + 1) * 512],
                                            in1=rcp[:, :], op=Alu.mult)
                    op = tlps.tile([128, 512], f32, tag="tl")
                    nc.tensor.matmul(out=op[0:64, :], lhsT=po_wT_sp, rhs=on[:, :],
                                     start=True, stop=True)
                    nc.vector.scalar_tensor_tensor(
                        out=R[0:64, j * 512 : (j + 1) * 512], in0=op[0:64, :],
                        scalar=po_b_hi, in1=R[0:64, j * 512 : (j + 1) * 512],
                        op0=Alu.add, op1=Alu.add,
                    )
                    if out_d is not None:
                        nc.sync.dma_start(
                            out=out_d[64:128, j * 512 : (j + 1) * 512],
                            in_=R[0:64, j * 512 : (j + 1) * 512],
                        )

                avs = apool.tile([128, HWS], bf16, tag="avs")
                pend = dict(pend_in) if pend_in else {}
                if 0 not in pend and "pend0" in fc:
                    pend[0] = fc.pop("pend0")
                for i0 in range(2):
                    if i0 not in pend:
                        pend[i0] = issue_gather(idxw_i, i0)
                pend_next = {}
                P = None
                for i, (h, c) in enumerate(steps):
                    if c == 0:
                        P = ppool.tile([128, 2, HWS], bf16, tag="P")
                    Wt, Dw = pend.pop(i)
                    # y-interp: Y = Wt + (1-fy) * Dw
                    Y = wpool.tile([128, 4160], bf16, tag="Y")
                    nc.vector.tensor_scalar(out=Y[:, :], in0=Dw[:, :],
                                            scalar1=fr1[:, c * 2 : c * 2 + 1],
                                            scalar2=None, op0=Alu.mult)
                    if i + 2 < 8:
                        pend[i + 2] = issue_gather(idxw_i, i + 2)
                    elif next_fc is not None:
                        # prefetch next block's first gather pairs
                        while "idxw_i" not in next_fc:
                            if not feeder():
                                break
                        if "idxw_i" in next_fc:
                            pend_next[i - 6] = issue_gather(next_fc["idxw_i"], i - 6)
                    nc.vector.tensor_tensor(out=Y[:, :], in0=Y[:, :], in1=Wt[:, :], op=Alu.add)
                    Y3 = Y[:, :].rearrange("p (r q) -> p r q", q=65)
                    kh = k_b[h * 32 : h * 32 + 17, c * 128 : (c + 1) * 128]
                    d0, d1 = diags[c]
                    for k in range(4):
                        qk = qkps.tile([128, 1024], f32, tag="qk")
                        for hf in range(2):
                            mc = k * 2 + hf
                            nc.tensor.matmul(
                                out=qk[:, hf * 512 : (hf + 1) * 512], lhsT=kh,
                                rhs=q_b[h * 32 : h * 32 + 17, mc * 512 : (mc + 1) * 512],
                                start=True, stop=False, tile_position=(h * 32, 0),
                            )
                        for hf in range(2):
                            mc = k * 2 + hf
                            nc.tensor.matmul(
                                out=qk[:, hf * 512 : (hf + 1) * 512], lhsT=d0,
                                rhs=Y3[:, mc * 8 : (mc + 1) * 8, 0:64],
                                start=False, stop=False,
                            )
                        for hf in range(2):
                            mc = k * 2 + hf
                            nc.tensor.matmul(
                                out=qk[:, hf * 512 : (hf + 1) * 512], lhsT=d1,
                                rhs=Y3[:, mc * 8 : (mc + 1) * 8, 1:65],
                                start=False, stop=True,
                            )
                        nc.scalar.activation(
                            out=P[:, c, k * 1024 : (k + 1) * 1024], in_=qk[:, :],
                            func=Act.Exp, bias=zb[:, :],
                        )
                    feeder()
                    if c == 1:
                        # AV for this head (+ tail interleaved into head 3)
                        for pr in range(4):
                            a0 = tlps.tile([128, 512], f32, tag="tl")
                            a1 = tlps.tile([128, 512], f32, tag="tl")
                            mca, mcb = pr * 2, pr * 2 + 1
                            for cc in range(2):
                                lw = vT1[:, cc * 128 + h * 32 : cc * 128 + (h + 1) * 32]
                                nc.tensor.matmul(
                                    out=a0[0:32, :], lhsT=lw,
                                    rhs=P[:, cc, mca * 512 : (mca + 1) * 512],
                                    start=(cc == 0), stop=(cc == 1),
                                )
                                nc.tensor.matmul(
                                    out=a1[0:32, :], lhsT=lw,
                                    rhs=P[:, cc, mcb * 512 : (mcb + 1) * 512],
                                    start=(cc == 0), stop=(cc == 1),
                                )
                            nc.vector.tensor_copy(
                                out=avs[h * 32 : (h + 1) * 32, mca * 512 : (mca + 1) * 512],
                                in_=a0[0:32, :])
                            nc.vector.tensor_copy(
                                out=avs[h * 32 : (h + 1) * 32, mcb * 512 : (mcb + 1) * 512],
                                in_=a1[0:32, :])
                            if h == 3:
                                tail_a(pr * 2)
                                tail_a(pr * 2 + 1)
                                if pr >= 1:
                                    tail_b(pr * 2 - 2)
                                    tail_b(pr * 2 - 1)
                        feeder()
                tail_b(6)
                tail_b(7)
                return pend_next

            def make_feeder(gen):
                def feeder():
                    if gen is None:
                        return False
                    try:
                        next(gen)
                        return True
                    except StopIteration:
                        return False
                return feeder

            def drain(gen):
                for _ in gen:
                    pass

            # ======================= schedule =======================
            fc0 = {}
            g0 = front(0, None, kvT0_d, fc0, stream_d=xq1_d)
            next(g0)
            load_bulk()
            drain(g0)
            nc.sync.dma_start(out=o1_d[0:64, :], in_=xi1_d[0:64, :])
            nc.sync.dma_start(out=o2_d[0:64, :], in_=xi2_d[0:64, :])
            fc1 = {}
            g1 = front(1, xq2[0:64, :], kvT0_d, fc1)
            pend1 = attn(0, fc0, xr1, make_feeder(g1), out_d=o1_d,
                         next_fc=fc1)
            drain(g1)
            fc2 = {}
            g2 = front(2, xq2[0:64, :], kvT1_d, fc2)
            pend2 = attn(1, fc1, xr2, make_feeder(g2),
                         pend_in=pend1 or None, next_fc=fc2)
            drain(g2)
            attn(2, fc2, xr2, make_feeder(None), out_d=o2_d,
                 pend_in=pend2 or None)

    nc.compile()
    return nc


def _host_prep(inputs):
    """Build per-core in_maps. inputs: dict of full numpy arrays."""
    import ml_dtypes

    x0, x1, x2 = inputs["x0"], inputs["x1"], inputs["x2"]

    def spread_cols(m):
        # m: [64(in), 64(out)] -> [64(in), 128] with out col h*16+j at h*32+j
        out = np.zeros((m.shape[0], 128), m.dtype)
        for h in range(4):
            out[:, h * 32 : h * 32 + 16] = m[:, h * 16 : (h + 1) * 16]
        return out

    def spread_rows(v):
        # v: [64, k] -> [128, k] with row h*16+j at h*32+j
        out = np.zeros((128,) + v.shape[1:], v.dtype)
        for h in range(4):
            out[h * 32 : h * 32 + 16] = v[h * 16 : (h + 1) * 16]
        return out

    # weight pack bf16: [64, 3*128]  (spread pq_wT)
    wpf = np.zeros((64, 3 * 128), np.float32)
    for b in range(3):
        wpf[:, b * 128 : (b + 1) * 128] = spread_cols(inputs["pq_w"][b].T)
    wpb = np.zeros((65, 3 * 192), ml_dtypes.bfloat16)
    for b in range(3):
        o = b * 192
        pk = np.zeros((65, 128), np.float32)
        pk[0:64] = spread_cols(inputs["pk_w"][b].T * 0.25)
        for h in range(4):
            pk[64, h * 32 : h * 32 + 16] = inputs["pk_b"][b][h * 16 : (h + 1) * 16] * 0.25
        for h in range(4):
            pq_bh = inputs["pq_b"][b][h * 16 : (h + 1) * 16]
            pk[:, h * 32 + 16] = pk[:, h * 32 : h * 32 + 16] @ pq_bh
        wpb[:, o : o + 128] = pk.astype(ml_dtypes.bfloat16)
        wpb[:64, o + 128 : o + 192] = inputs["pv_w"][b].T.astype(ml_dtypes.bfloat16)
        wpb[64, o + 128 : o + 192] = inputs["pv_b"][b].astype(ml_dtypes.bfloat16)
    # const pack [128, 598]
    cp = np.zeros((128, 599), np.float32)
    for h in range(4):
        cp[h * 32 + 16, 598] = 1.0
    cp[:, 0:128] = np.eye(128, dtype=np.float32)
    ys = (np.linspace(0.5, HK - 0.5, HK) / (HK - 1.0)) * 2.0 - 1.0
    cp[0, 128:384] = np.repeat(ys, WK)         # y per n (i-major)
    cp[1, 128:384] = np.tile(ys, HK)           # x per n
    cp[0, 384:512] = 1.0                       # ones1_128
    for h in range(4):
        cp[h * 32 : h * 32 + 16, 520] = 1.0 / 64.0
    for b in range(3):
        cp[:, 521 + b] = spread_rows(inputs["pq_b"][b][:, None])[:, 0]
        cp[0:64, 524 + b] = inputs["po_b"][b]
        bc0 = 527 + b * 21
        cp[:, bc0 : bc0 + 16] = spread_rows(inputs["dw_w"][b].reshape(64, 16))
        dwb_eff = (inputs["dw_b"][b]
                   + inputs["dw_w"][b].reshape(64, 16).sum(1) * inputs["pq_b"][b])
        cp[:, bc0 + 16] = spread_rows(dwb_eff[:, None])[:, 0]
        cp[:, bc0 + 17] = spread_rows(inputs["ln_g"][b][:, None])[:, 0]
        cp[:, bc0 + 18] = spread_rows(inputs["ln_b"][b][:, None])[:, 0]
        cp[:, bc0 + 19 : bc0 + 21] = spread_rows(inputs["pw_w"][b].T)
    cp[:, 590] = 0.0
    cp[:, 591] = 1.0
    cp[:, 592] = 64.0
    cp[:, 593] = 65.0
    for h in range(4):
        cp[:, 594 + h] = float(h * THEAD)
    cpb = np.zeros((128, 320), ml_dtypes.bfloat16)
    b4 = np.zeros((128, 128), np.float32)
    for h in range(4):
        b4[h * 32 + 16, h * 32 : (h + 1) * 32] = 1.0
    cpb[:, 0:128] = b4.astype(ml_dtypes.bfloat16)
    for b in range(3):
        poT = inputs["po_w"][b].T  # [c, o]
        for h in range(4):
            cpb[h * 32 : h * 32 + 16, 128 + b * 64 : 128 + (b + 1) * 64] = poT[
                h * 16 : (h + 1) * 16
            ].astype(ml_dtypes.bfloat16)
    # rpe slice tables bf16: T windows then D (row-diff) windows
    tab = np.zeros((2, NBLK, NH, 64, TROW, TCOL), ml_dtypes.bfloat16)
    rpe = inputs["rpe"]
    for b in range(3):
        for h in range(4):
            pad = np.zeros((129, 128), np.float32)
            pad[0:127, 0:127] = rpe[b, h]
            dif = pad[1:129] - pad[0:128]
            for x0s in range(64):
                tab[0, b, h, x0s] = pad[0:128, x0s : x0s + 65].astype(ml_dtypes.bfloat16)
                tab[1, b, h, x0s] = dif[:, x0s : x0s + 65].astype(ml_dtypes.bfloat16)
    tab = tab.reshape(-1, 1)

    in_maps = []
    for bb in range(B):
        m = {
            "xi1": np.ascontiguousarray(x1[bb].reshape(C, HWS)),
            "xi2": np.ascontiguousarray(x2[bb].reshape(C, HWS)),
            "kvT0": np.ascontiguousarray(x0[bb, :64].reshape(64, HWS).T),
            "xq1": np.ascontiguousarray(x1[bb, :64].reshape(64, HWS)),
            "xq2": np.ascontiguousarray(x2[bb, :64].reshape(64, HWS)),
            "kvT1": np.ascontiguousarray(x1[bb, :64].reshape(64, HWS).T),
            "wpf": wpf,
            "wpb": wpb,
            "cp": cp,
            "cpb": cpb,
            "rpetab": tab,
        }
        in_maps.append(m)
    return in_maps


def kernel(**inputs):
    from concourse.bass_utils import run_bass_kernel_spmd

    if "nc" not in _CACHE:
        _CACHE["nc"] = _build_graph()
    nc = _CACHE["nc"]
    in_maps = _host_prep(inputs)
    res = run_bass_kernel_spmd(nc, in_maps, core_ids=list(range(8)))
    out = np.zeros((NBLK, B, C, H, W), np.float32)
    out[0] = inputs["x0"]
    for bb in range(B):
        out[1, bb] = res.results[bb]["o1"].reshape(C, H, W)
        out[2, bb] = res.results[bb]["o2"].reshape(C, H, W)
    return out
